# revision 3
# baseline (speedup 1.0000x reference)
"""ComplexMoELayer TRN2 kernel: dense expert-parallel across 8 NeuronCores.

Layout: everything on-device is [feature, token] ("option B"):
  - host feeds x^T [D=512, NT=2048] fp32 (both real/imag)
  - L1: h[m-tile] = sum_k W1[k,m].T @ xT[k]  -> PSUM [128, CH]
  - ComplexModReLU on PSUM tiles, emit bf16 h tiles for L2
  - L2: o[m4]  = sum_k W2[k,m4].T @ h[k]     -> PSUM [128, CH]
  - out = (o + b2) * w_token  (w = top1 routing weight, 0 for foreign tokens)
Host sums the 8 per-core partial outputs (disjoint support) and transposes back.

Gating runs in fp32 (routing argmax needs ~1e-4 accuracy; min top-2 gap of the
score distribution is ~2.5e-4):  amp = sqrt(xr^2+xi^2),
phase = 2*atan(xi/(amp+xr)),  scores^T = gate_W^T @ [amp;phase]^T.
Per-core gate_W columns are permuted so that "my expert" is always index 0,
keeping the program SPMD-identical across cores.
"""

import numpy as np

import concourse.bass as bass
import concourse.mybir as mybir
import concourse.tile as tile
from concourse import bacc
from concourse.bass_utils import run_bass_kernel_spmd
from concourse.masks import make_identity

F32 = mybir.dt.float32
BF16 = mybir.dt.bfloat16
AF = mybir.ActivationFunctionType
ALU = mybir.AluOpType

E, D, H = 8, 512, 2048
B, S = 4, 512
NT = B * S            # 2048 tokens
CH = 512              # tokens per chunk
NCH = NT // CH        # 4 chunks
KD = D // 128         # 4  k-tiles over D
KH = H // 128         # 16 k-tiles over H
MD = D // 128         # 4  m-tiles of output D
EPS = 1e-10

_CACHE: dict = {}
LAST_RESULT = None    # test harness reads exec_time_ns from here


def _build_nc():
    nc = bacc.Bacc("TRN2", target_bir_lowering=False, debug=False)

    xrT = nc.dram_tensor("xrT", [D, NT], F32, kind="ExternalInput")
    xiT = nc.dram_tensor("xiT", [D, NT], F32, kind="ExternalInput")
    gWp = nc.dram_tensor("gWp", [128, 8, 8], F32, kind="ExternalInput")
    gb = nc.dram_tensor("gb", [8, 1], F32, kind="ExternalInput")
    b1r_d = nc.dram_tensor("b1r", [128, KH], F32, kind="ExternalInput")
    b1i_d = nc.dram_tensor("b1i", [128, KH], F32, kind="ExternalInput")
    modb_d = nc.dram_tensor("modb", [128, KH], F32, kind="ExternalInput")
    b2r_d = nc.dram_tensor("b2r", [128, MD], F32, kind="ExternalInput")
    b2i_d = nc.dram_tensor("b2i", [128, MD], F32, kind="ExternalInput")
    W1r_d = nc.dram_tensor("W1r", [D, H], F32, kind="ExternalInput")
    W1i_d = nc.dram_tensor("W1i", [D, H], F32, kind="ExternalInput")
    W2r_d = nc.dram_tensor("W2r", [H, D], F32, kind="ExternalInput")
    W2i_d = nc.dram_tensor("W2i", [H, D], F32, kind="ExternalInput")
    out_r = nc.dram_tensor("out_r", [D, NT], F32, kind="ExternalOutput")
    out_i = nc.dram_tensor("out_i", [D, NT], F32, kind="ExternalOutput")
    w_scr = nc.dram_tensor("w_scr", [KH, 128], F32)  # internal scratch for w rows

    with tile.TileContext(nc) as tc:
        import contextlib

        ctx = contextlib.ExitStack()
        with ctx:
            smalls = ctx.enter_context(tc.tile_pool(name="smalls", bufs=1))
            wload = ctx.enter_context(tc.tile_pool(name="wload", bufs=2))
            wbf = ctx.enter_context(tc.tile_pool(name="wbf", bufs=1))
            xf = ctx.enter_context(tc.tile_pool(name="xf", bufs=2))
            xb = ctx.enter_context(tc.tile_pool(name="xb", bufs=1))
            tmp = ctx.enter_context(tc.tile_pool(name="tmp", bufs=2))
            hp = ctx.enter_context(tc.tile_pool(name="hp", bufs=1))
            op = ctx.enter_context(tc.tile_pool(name="op", bufs=2))
            wbc = ctx.enter_context(tc.tile_pool(name="wbc", bufs=1))
            scp = ctx.enter_context(tc.tile_pool(name="scp", bufs=2))
            pp = ctx.enter_context(tc.tile_pool(name="pp", bufs=2, space="PSUM"))

            # ---- small constants ----
            gw_sb = smalls.tile([128, 8, 8], F32)
            nc.sync.dma_start(out=gw_sb, in_=gWp[:])
            gb_sb = smalls.tile([8, 1], F32)
            nc.sync.dma_start(out=gb_sb, in_=gb[:])
            b1r_sb = smalls.tile([128, KH], F32)
            nc.sync.dma_start(out=b1r_sb, in_=b1r_d[:])
            b1i_sb = smalls.tile([128, KH], F32)
            nc.sync.dma_start(out=b1i_sb, in_=b1i_d[:])
            modb_sb = smalls.tile([128, KH], F32)
            nc.sync.dma_start(out=modb_sb, in_=modb_d[:])
            b2r_sb = smalls.tile([128, MD], F32)
            nc.sync.dma_start(out=b2r_sb, in_=b2r_d[:])
            b2i_sb = smalls.tile([128, MD], F32)
            nc.sync.dma_start(out=b2i_sb, in_=b2i_d[:])
            ident = smalls.tile([128, 128], F32)
            make_identity(nc, ident)
            eps_sb = smalls.tile([128, 1], F32)
            nc.vector.memset(eps_sb, EPS)
            scores_t = smalls.tile([128, KH, 8], F32)
            e_t = smalls.tile([128, KH, 8], F32)
            mx = smalls.tile([128, KH], F32)
            sm = smalls.tile([128, KH], F32)
            rs = smalls.tile([128, KH], F32)
            pe = smalls.tile([128, KH], F32)
            msk = smalls.tile([128, KH], F32)
            w_pt = smalls.tile([128, KH], F32)
            w16 = smalls.tile([KH, 128], F32)

            # ---- expert weights: DMA fp32 -> cast bf16 (resident) ----
            w1r_bf, w1i_bf = [], []
            for k in range(KD):
                t_r = wbf.tile([128, H], BF16, tag=f"w1r{k}")
                t_i = wbf.tile([128, H], BF16, tag=f"w1i{k}")
                for c4 in range(4):
                    sl = bass.ts(c4, 512)
                    wt = wload.tile([128, 512], F32, tag="wl")
                    nc.sync.dma_start(out=wt, in_=W1r_d[k * 128:(k + 1) * 128, sl])
                    nc.vector.tensor_copy(out=t_r[:, sl], in_=wt)
                    wt2 = wload.tile([128, 512], F32, tag="wl")
                    nc.sync.dma_start(out=wt2, in_=W1i_d[k * 128:(k + 1) * 128, sl])
                    nc.vector.tensor_copy(out=t_i[:, sl], in_=wt2)
                w1r_bf.append(t_r)
                w1i_bf.append(t_i)
            w2r_bf, w2i_bf = [], []
            for k in range(KH):
                wt = wload.tile([128, 512], F32, tag="wl")
                nc.sync.dma_start(out=wt, in_=W2r_d[k * 128:(k + 1) * 128, :])
                t_r = wbf.tile([128, D], BF16, tag=f"w2r{k}")
                nc.scalar.copy(out=t_r, in_=wt)
                wt2 = wload.tile([128, 512], F32, tag="wl")
                nc.sync.dma_start(out=wt2, in_=W2i_d[k * 128:(k + 1) * 128, :])
                t_i = wbf.tile([128, D], BF16, tag=f"w2i{k}")
                nc.scalar.copy(out=t_i, in_=wt2)
                w2r_bf.append(t_r)
                w2i_bf.append(t_i)

            # ---- phase 0: gating over all chunks ----
            for t in range(NCH):
                tok = bass.ts(t, CH)
                sc_ps = pp.tile([8, CH], F32, tag="or")
                for p in range(KD):
                    xr = xf.tile([128, CH], F32, tag=f"xr{p}")
                    nc.sync.dma_start(out=xr, in_=xrT[p * 128:(p + 1) * 128, tok])
                    xi = xf.tile([128, CH], F32, tag=f"xi{p}")
                    nc.sync.dma_start(out=xi, in_=xiT[p * 128:(p + 1) * 128, tok])
                    v = tmp.tile([128, CH], F32, tag="tA")
                    nc.scalar.activation(out=v, in_=xr, func=AF.Square)
                    v2 = tmp.tile([128, CH], F32, tag="tB")
                    nc.scalar.activation(out=v2, in_=xi, func=AF.Square)
                    nc.gpsimd.tensor_tensor(out=v, in0=v, in1=v2, op=ALU.add)
                    amp = tmp.tile([128, CH], F32, tag="tC")
                    nc.scalar.activation(out=amp, in_=v, func=AF.Sqrt)
                    # half-angle atan2: ph = 2*atan(xi / (amp + xr))
                    nc.gpsimd.tensor_tensor(out=v, in0=amp, in1=xr, op=ALU.add)
                    nc.vector.reciprocal(out=v2, in_=v)
                    nc.vector.tensor_tensor(out=v, in0=xi, in1=v2, op=ALU.mult)
                    nc.scalar.activation(out=v, in_=v, func=AF.Arctan)
                    ph = tmp.tile([128, CH], F32, tag="tD")
                    nc.vector.tensor_scalar(
                        out=ph, in0=v, scalar1=2.0, scalar2=None, op0=ALU.mult
                    )
                    nc.tensor.matmul(
                        sc_ps, gw_sb[:, p, :], amp, start=(p == 0), stop=False
                    )
                    nc.tensor.matmul(
                        sc_ps, gw_sb[:, KD + p, :], ph, start=False, stop=(p == KD - 1)
                    )
                sc_sb = scp.tile([8, CH], F32, tag="sc")
                nc.vector.tensor_scalar(
                    out=sc_sb, in0=sc_ps, scalar1=gb_sb[:, 0:1], scalar2=None,
                    op0=ALU.add,
                )
                for g4 in range(4):
                    tp_ps = pp.tile([128, 8], F32, tag="oi")
                    nc.tensor.transpose(
                        tp_ps, sc_sb[:, g4 * 128:(g4 + 1) * 128], ident[0:8, 0:8]
                    )
                    nc.scalar.copy(out=scores_t[:, t * 4 + g4, :], in_=tp_ps)

            # ---- softmax / top-1 weights (expert 0 = this core's expert) ----
            nc.scalar.activation(out=e_t, in_=scores_t, func=AF.Exp)
            nc.vector.tensor_reduce(
                out=mx, in_=scores_t, axis=mybir.AxisListType.X, op=ALU.max
            )
            nc.vector.tensor_reduce(
                out=sm, in_=e_t, axis=mybir.AxisListType.X, op=ALU.add
            )
            nc.vector.reciprocal(out=rs, in_=sm)
            nc.vector.tensor_tensor(out=pe, in0=e_t[:, :, 0], in1=rs, op=ALU.mult)
            nc.vector.tensor_tensor(
                out=msk, in0=scores_t[:, :, 0], in1=mx, op=ALU.is_ge
            )
            nc.vector.tensor_tensor(out=w_pt, in0=pe, in1=msk, op=ALU.mult)
            wt_ps = pp.tile([KH, 128], F32, tag="oi")
            nc.tensor.transpose(wt_ps, w_pt, ident)
            nc.scalar.copy(out=w16, in_=wt_ps)
            nc.sync.dma_start(out=w_scr[:], in_=w16)
            wb_tiles = []
            for t in range(NCH):
                wb_t = wbc.tile([128, CH], F32, tag=f"wb{t}")
                for g4 in range(4):
                    g = t * 4 + g4
                    row = w_scr[g:g + 1, :]
                    bcast = bass.AP(
                        tensor=row.tensor, offset=row.offset,
                        ap=[[0, 128]] + list(row.ap[1:]),
                    )
                    nc.sync.dma_start(
                        out=wb_t[:, g4 * 128:(g4 + 1) * 128], in_=bcast
                    )
                wb_tiles.append(wb_t)

            # ---- phase E: experts, chunk by chunk ----
            for t in range(NCH):
                tok = bass.ts(t, CH)
                xrb, xib, xnb = [], [], []
                for p in range(KD):
                    xrf = xf.tile([128, CH], F32, tag=f"xr{p}")
                    nc.sync.dma_start(out=xrf, in_=xrT[p * 128:(p + 1) * 128, tok])
                    xif = xf.tile([128, CH], F32, tag=f"xi{p}")
                    nc.sync.dma_start(out=xif, in_=xiT[p * 128:(p + 1) * 128, tok])
                    xr_b = xb.tile([128, CH], BF16, tag=f"xrb{p}")
                    nc.vector.tensor_copy(out=xr_b, in_=xrf)
                    xi_b = xb.tile([128, CH], BF16, tag=f"xib{p}")
                    nc.vector.tensor_copy(out=xi_b, in_=xif)
                    xn_b = xb.tile([128, CH], BF16, tag=f"xnb{p}")
                    nc.vector.tensor_scalar(
                        out=xn_b, in0=xif, scalar1=-1.0, scalar2=None, op0=ALU.mult
                    )
                    xrb.append(xr_b)
                    xib.append(xi_b)
                    xnb.append(xn_b)

                hrb, hib, hnb = [], [], []
                for m in range(KH):
                    msl = bass.ts(m, 128)
                    ps_hr = pp.tile([128, CH], F32, tag="hr")
                    for k in range(KD):
                        nc.tensor.matmul(
                            ps_hr, w1r_bf[k][:, msl], xrb[k],
                            start=(k == 0), stop=False,
                        )
                        nc.tensor.matmul(
                            ps_hr, w1i_bf[k][:, msl], xnb[k],
                            start=False, stop=(k == KD - 1),
                        )
                    ps_hi = pp.tile([128, CH], F32, tag="hi")
                    for k in range(KD):
                        nc.tensor.matmul(
                            ps_hi, w1i_bf[k][:, msl], xrb[k],
                            start=(k == 0), stop=False,
                        )
                        nc.tensor.matmul(
                            ps_hi, w1r_bf[k][:, msl], xib[k],
                            start=False, stop=(k == KD - 1),
                        )
                    # ComplexModReLU
                    b1r_m = b1r_sb[:, m:m + 1]
                    b1i_m = b1i_sb[:, m:m + 1]
                    mb_m = modb_sb[:, m:m + 1]
                    v1 = tmp.tile([128, CH], F32, tag="tA")
                    nc.scalar.activation(out=v1, in_=ps_hr, func=AF.Square, bias=b1r_m)
                    v2 = tmp.tile([128, CH], F32, tag="tB")
                    nc.scalar.activation(out=v2, in_=ps_hi, func=AF.Square, bias=b1i_m)
                    nc.gpsimd.tensor_tensor(out=v1, in0=v1, in1=v2, op=ALU.add)
                    nc.scalar.activation(out=v1, in_=v1, func=AF.Sqrt, bias=eps_sb)
                    nc.scalar.activation(out=v2, in_=v1, func=AF.Relu, bias=mb_m)
                    q = tmp.tile([128, CH], F32, tag="tC")
                    nc.vector.reciprocal(out=q, in_=v1)
                    nc.vector.tensor_tensor(out=v2, in0=v2, in1=q, op=ALU.mult)
                    h_r = hp.tile([128, CH], BF16, tag=f"hr{m}")
                    nc.vector.scalar_tensor_tensor(
                        out=h_r, in0=ps_hr, scalar=b1r_m, in1=v2,
                        op0=ALU.add, op1=ALU.mult,
                    )
                    h_i = hp.tile([128, CH], BF16, tag=f"hi{m}")
                    nc.vector.scalar_tensor_tensor(
                        out=h_i, in0=ps_hi, scalar=b1i_m, in1=v2,
                        op0=ALU.add, op1=ALU.mult,
                    )
                    h_n = hp.tile([128, CH], BF16, tag=f"hn{m}")
                    nc.vector.tensor_scalar(
                        out=h_n, in0=h_i, scalar1=-1.0, scalar2=None, op0=ALU.mult
                    )
                    hrb.append(h_r)
                    hib.append(h_i)
                    hnb.append(h_n)

                for m4 in range(MD):
                    msl = bass.ts(m4, 128)
                    ps_or = pp.tile([128, CH], F32, tag="or")
                    for k in range(KH):
                        nc.tensor.matmul(
                            ps_or, w2r_bf[k][:, msl], hrb[k],
                            start=(k == 0), stop=False,
                        )
                        nc.tensor.matmul(
                            ps_or, w2i_bf[k][:, msl], hnb[k],
                            start=False, stop=(k == KH - 1),
                        )
                    ps_oi = pp.tile([128, CH], F32, tag="oi")
                    for k in range(KH):
                        nc.tensor.matmul(
                            ps_oi, w2i_bf[k][:, msl], hrb[k],
                            start=(k == 0), stop=False,
                        )
                        nc.tensor.matmul(
                            ps_oi, w2r_bf[k][:, msl], hib[k],
                            start=False, stop=(k == KH - 1),
                        )
                    o_r = op.tile([128, CH], F32, tag="osr")
                    nc.vector.scalar_tensor_tensor(
                        out=o_r, in0=ps_or, scalar=b2r_sb[:, m4:m4 + 1],
                        in1=wb_tiles[t], op0=ALU.add, op1=ALU.mult,
                    )
                    nc.sync.dma_start(
                        out=out_r[m4 * 128:(m4 + 1) * 128, tok], in_=o_r
                    )
                    o_i = op.tile([128, CH], F32, tag="osi")
                    nc.vector.scalar_tensor_tensor(
                        out=o_i, in0=ps_oi, scalar=b2i_sb[:, m4:m4 + 1],
                        in1=wb_tiles[t], op0=ALU.add, op1=ALU.mult,
                    )
                    nc.sync.dma_start(
                        out=out_i[m4 * 128:(m4 + 1) * 128, tok], in_=o_i
                    )

    nc.compile()
    return nc


def kernel(**inputs):
    global LAST_RESULT
    f32 = lambda a: np.ascontiguousarray(np.asarray(a, dtype=np.float32))
    xr = f32(inputs["x_real"]).reshape(NT, D).T.copy()
    xi = f32(inputs["x_imag"]).reshape(NT, D).T.copy()
    gW = f32(inputs["gate_W"])
    gb = f32(inputs["gate_b"])
    W1r, W1i = f32(inputs["W1r"]), f32(inputs["W1i"])
    W2r, W2i = f32(inputs["W2r"]), f32(inputs["W2i"])
    b1r, b1i = f32(inputs["b1r"]), f32(inputs["b1i"])
    modb = f32(inputs["mod_b"])
    b2r, b2i = f32(inputs["b2r"]), f32(inputs["b2i"])

    if "nc" not in _CACHE:
        _CACHE["nc"] = _build_nc()
    nc = _CACHE["nc"]

    in_maps = []
    for c in range(E):
        perm = [c] + [e for e in range(E) if e != c]
        gWp = np.ascontiguousarray(
            gW[:, perm].reshape(8, 128, 8).transpose(1, 0, 2)
        )
        in_maps.append({
            "xrT": xr, "xiT": xi,
            "gWp": gWp,
            "gb": np.ascontiguousarray(gb[perm].reshape(8, 1)),
            "b1r": np.ascontiguousarray(b1r[c].reshape(KH, 128).T),
            "b1i": np.ascontiguousarray(b1i[c].reshape(KH, 128).T),
            "modb": np.ascontiguousarray(modb[c].reshape(KH, 128).T),
            "b2r": np.ascontiguousarray(b2r[c].reshape(MD, 128).T),
            "b2i": np.ascontiguousarray(b2i[c].reshape(MD, 128).T),
            "W1r": np.ascontiguousarray(W1r[c]),
            "W1i": np.ascontiguousarray(W1i[c]),
            "W2r": np.ascontiguousarray(W2r[c]),
            "W2i": np.ascontiguousarray(W2i[c]),
        })

    res = run_bass_kernel_spmd(nc, in_maps, list(range(E)))
    LAST_RESULT = res
    acc_r = np.zeros((D, NT), np.float32)
    acc_i = np.zeros((D, NT), np.float32)
    for c in range(E):
        acc_r += res.results[c]["out_r"]
        acc_i += res.results[c]["out_i"]
    out_r = np.ascontiguousarray(acc_r.T).reshape(B, S, D)
    out_i = np.ascontiguousarray(acc_i.T).reshape(B, S, D)
    return out_r, out_i


# revision 7
# speedup vs baseline: 1.0777x; 1.0777x over previous
"""ComplexMoELayer TRN2 kernel: dense expert-parallel across 8 NeuronCores.

Layout: everything on-device is [feature, token] ("option B"):
  - host feeds x^T [D=512, NT=2048] fp32 (both real/imag)
  - L1: h[m-tile] = sum_k W1[k,m].T @ xT[k]  -> PSUM [128, CH]
  - ComplexModReLU on PSUM tiles, emit bf16 h tiles for L2
  - L2: o[m4]  = sum_k W2[k,m4].T @ h[k]     -> PSUM [128, CH]
  - out = (o + b2) * w_token  (w = top1 routing weight, 0 for foreign tokens)
Host sums the 8 per-core partial outputs (disjoint support) and transposes back.

Gating runs in fp32 (routing argmax needs ~1e-4 accuracy; min top-2 gap of the
score distribution is ~2.5e-4):  amp = sqrt(xr^2+xi^2),
phase = 2*atan(xi/(amp+xr)),  scores^T = gate_W^T @ [amp;phase]^T.
Per-core gate_W columns are permuted so that "my expert" is always index 0,
keeping the program SPMD-identical across cores.
"""

import numpy as np

import concourse.bass as bass
import concourse.mybir as mybir
import concourse.tile as tile
from concourse import bacc
from concourse.bass_utils import run_bass_kernel_spmd
from concourse.masks import make_identity

F32 = mybir.dt.float32
BF16 = mybir.dt.bfloat16
AF = mybir.ActivationFunctionType
ALU = mybir.AluOpType

E, D, H = 8, 512, 2048
B, S = 4, 512
NT = B * S            # 2048 tokens
CH = 512              # tokens per chunk
NCH = NT // CH        # 4 chunks
KD = D // 128         # 4  k-tiles over D
KH = H // 128         # 16 k-tiles over H
MD = D // 128         # 4  m-tiles of output D
EPS = 1e-10

_CACHE: dict = {}
LAST_RESULT = None    # test harness reads exec_time_ns from here


def _build_nc():
    nc = bacc.Bacc("TRN2", target_bir_lowering=False, debug=False)

    xrT = nc.dram_tensor("xrT", [D, NT], F32, kind="ExternalInput")
    xiT = nc.dram_tensor("xiT", [D, NT], F32, kind="ExternalInput")
    gWp = nc.dram_tensor("gWp", [128, 8, 8], F32, kind="ExternalInput")
    gb = nc.dram_tensor("gb", [8, 1], F32, kind="ExternalInput")
    b1r_d = nc.dram_tensor("b1r", [128, KH], F32, kind="ExternalInput")
    b1i_d = nc.dram_tensor("b1i", [128, KH], F32, kind="ExternalInput")
    modb_d = nc.dram_tensor("modb", [128, KH], F32, kind="ExternalInput")
    b2r_d = nc.dram_tensor("b2r", [128, MD], F32, kind="ExternalInput")
    b2i_d = nc.dram_tensor("b2i", [128, MD], F32, kind="ExternalInput")
    W1r_d = nc.dram_tensor("W1r", [D, H], F32, kind="ExternalInput")
    W1i_d = nc.dram_tensor("W1i", [D, H], F32, kind="ExternalInput")
    W2r_d = nc.dram_tensor("W2r", [H, D], F32, kind="ExternalInput")
    W2i_d = nc.dram_tensor("W2i", [H, D], F32, kind="ExternalInput")
    out_r = nc.dram_tensor("out_r", [D, NT], F32, kind="ExternalOutput")
    out_i = nc.dram_tensor("out_i", [D, NT], F32, kind="ExternalOutput")
    w_scr = nc.dram_tensor("w_scr", [KH, 128], F32)  # internal scratch for w rows

    with tile.TileContext(nc) as tc:
        import contextlib

        ctx = contextlib.ExitStack()
        with ctx:
            smalls = ctx.enter_context(tc.tile_pool(name="smalls", bufs=1))
            wload = ctx.enter_context(tc.tile_pool(name="wload", bufs=2))
            wbf = ctx.enter_context(tc.tile_pool(name="wbf", bufs=1))
            xf = ctx.enter_context(tc.tile_pool(name="xf", bufs=2))
            xb = ctx.enter_context(tc.tile_pool(name="xb", bufs=1))
            tmp = ctx.enter_context(tc.tile_pool(name="tmp", bufs=2))
            hp = ctx.enter_context(tc.tile_pool(name="hp", bufs=1))
            op = ctx.enter_context(tc.tile_pool(name="op", bufs=2))
            wbc = ctx.enter_context(tc.tile_pool(name="wbc", bufs=1))
            scp = ctx.enter_context(tc.tile_pool(name="scp", bufs=2))
            pp = ctx.enter_context(tc.tile_pool(name="pp", bufs=2, space="PSUM"))

            # ---- small constants ----
            gw_sb = smalls.tile([128, 8, 8], F32)
            nc.sync.dma_start(out=gw_sb, in_=gWp[:])
            gb_sb = smalls.tile([8, 1], F32)
            nc.sync.dma_start(out=gb_sb, in_=gb[:])
            b1r_sb = smalls.tile([128, KH], F32)
            nc.sync.dma_start(out=b1r_sb, in_=b1r_d[:])
            b1i_sb = smalls.tile([128, KH], F32)
            nc.sync.dma_start(out=b1i_sb, in_=b1i_d[:])
            modb_sb = smalls.tile([128, KH], F32)
            nc.sync.dma_start(out=modb_sb, in_=modb_d[:])
            b2r_sb = smalls.tile([128, MD], F32)
            nc.sync.dma_start(out=b2r_sb, in_=b2r_d[:])
            b2i_sb = smalls.tile([128, MD], F32)
            nc.sync.dma_start(out=b2i_sb, in_=b2i_d[:])
            ident = smalls.tile([128, 128], F32)
            make_identity(nc, ident)
            eps_sb = smalls.tile([128, 1], F32)
            nc.vector.memset(eps_sb, EPS)
            scores_t = smalls.tile([128, KH, 8], F32)
            e_t = smalls.tile([128, KH, 8], F32)
            mx = smalls.tile([128, KH], F32)
            sm = smalls.tile([128, KH], F32)
            rs = smalls.tile([128, KH], F32)
            pe = smalls.tile([128, KH], F32)
            msk = smalls.tile([128, KH], F32)
            w_pt = smalls.tile([128, KH], F32)
            w16 = smalls.tile([KH, 128], F32)

            # ---- expert weights: DMA fp32 -> cast bf16 (resident) ----
            w1r_bf, w1i_bf = [], []
            for k in range(KD):
                t_r = wbf.tile([128, H], BF16, tag=f"w1r{k}")
                t_i = wbf.tile([128, H], BF16, tag=f"w1i{k}")
                for c4 in range(4):
                    sl = bass.ts(c4, 512)
                    wt = wload.tile([128, 512], F32, tag="wl")
                    nc.sync.dma_start(out=wt, in_=W1r_d[k * 128:(k + 1) * 128, sl])
                    nc.vector.tensor_copy(out=t_r[:, sl], in_=wt)
                    wt2 = wload.tile([128, 512], F32, tag="wl")
                    nc.sync.dma_start(out=wt2, in_=W1i_d[k * 128:(k + 1) * 128, sl])
                    nc.vector.tensor_copy(out=t_i[:, sl], in_=wt2)
                w1r_bf.append(t_r)
                w1i_bf.append(t_i)
            w2r_bf, w2i_bf = [], []
            for k in range(KH):
                wt = wload.tile([128, 512], F32, tag="wl")
                nc.sync.dma_start(out=wt, in_=W2r_d[k * 128:(k + 1) * 128, :])
                t_r = wbf.tile([128, D], BF16, tag=f"w2r{k}")
                nc.scalar.copy(out=t_r, in_=wt)
                wt2 = wload.tile([128, 512], F32, tag="wl")
                nc.sync.dma_start(out=wt2, in_=W2i_d[k * 128:(k + 1) * 128, :])
                t_i = wbf.tile([128, D], BF16, tag=f"w2i{k}")
                nc.scalar.copy(out=t_i, in_=wt2)
                w2r_bf.append(t_r)
                w2i_bf.append(t_i)

            # ---- phase 0: gating over all chunks ----
            for t in range(NCH):
                tok = bass.ts(t, CH)
                sc_ps = pp.tile([8, CH], F32, tag="or")
                for p in range(KD):
                    xr = xf.tile([128, CH], F32, tag=f"xr{p}")
                    nc.sync.dma_start(out=xr, in_=xrT[p * 128:(p + 1) * 128, tok])
                    xi = xf.tile([128, CH], F32, tag=f"xi{p}")
                    nc.sync.dma_start(out=xi, in_=xiT[p * 128:(p + 1) * 128, tok])
                    v = tmp.tile([128, CH], F32, tag="tA")
                    nc.scalar.activation(out=v, in_=xr, func=AF.Square)
                    v2 = tmp.tile([128, CH], F32, tag="tB")
                    nc.scalar.activation(out=v2, in_=xi, func=AF.Square)
                    nc.gpsimd.tensor_tensor(out=v, in0=v, in1=v2, op=ALU.add)
                    amp = tmp.tile([128, CH], F32, tag="tC")
                    nc.scalar.activation(out=amp, in_=v, func=AF.Sqrt)
                    # half-angle atan2: ph = 2*atan(xi / (amp + xr))
                    nc.gpsimd.tensor_tensor(out=v, in0=amp, in1=xr, op=ALU.add)
                    # d can round to exactly 0 (xr<0, |xi|<<|xr|): clamp so the
                    # seeded reciprocal stays defined; atan then saturates to
                    # +-pi/2 and phase to +-pi, matching arctan2 to ~1e-4.
                    nc.gpsimd.tensor_scalar(
                        out=v, in0=v, scalar1=1e-30, scalar2=None, op0=ALU.max
                    )
                    nc.vector.reciprocal_approx_fast(out=v2, in_=v)
                    nc.vector.tensor_tensor(out=v, in0=xi, in1=v2, op=ALU.mult)
                    nc.scalar.activation(out=v, in_=v, func=AF.Arctan)
                    ph = tmp.tile([128, CH], F32, tag="tD")
                    nc.vector.tensor_scalar(
                        out=ph, in0=v, scalar1=2.0, scalar2=None, op0=ALU.mult
                    )
                    nc.tensor.matmul(
                        sc_ps, gw_sb[:, p, :], amp, start=(p == 0), stop=False
                    )
                    nc.tensor.matmul(
                        sc_ps, gw_sb[:, KD + p, :], ph, start=False, stop=(p == KD - 1)
                    )
                sc_sb = scp.tile([8, CH], F32, tag="sc")
                nc.vector.tensor_scalar(
                    out=sc_sb, in0=sc_ps, scalar1=gb_sb[:, 0:1], scalar2=None,
                    op0=ALU.add,
                )
                for g4 in range(4):
                    tp_ps = pp.tile([128, 8], F32, tag="oi")
                    nc.tensor.transpose(
                        tp_ps, sc_sb[:, g4 * 128:(g4 + 1) * 128], ident[0:8, 0:8]
                    )
                    nc.scalar.copy(out=scores_t[:, t * 4 + g4, :], in_=tp_ps)

            # ---- softmax / top-1 weights (expert 0 = this core's expert) ----
            nc.scalar.activation(out=e_t, in_=scores_t, func=AF.Exp)
            nc.vector.tensor_reduce(
                out=mx, in_=scores_t, axis=mybir.AxisListType.X, op=ALU.max
            )
            nc.vector.tensor_reduce(
                out=sm, in_=e_t, axis=mybir.AxisListType.X, op=ALU.add
            )
            nc.vector.reciprocal_approx_fast(out=rs, in_=sm)
            nc.vector.tensor_tensor(out=pe, in0=e_t[:, :, 0], in1=rs, op=ALU.mult)
            nc.vector.tensor_tensor(
                out=msk, in0=scores_t[:, :, 0], in1=mx, op=ALU.is_ge
            )
            nc.vector.tensor_tensor(out=w_pt, in0=pe, in1=msk, op=ALU.mult)
            wt_ps = pp.tile([KH, 128], F32, tag="oi")
            nc.tensor.transpose(wt_ps, w_pt, ident)
            nc.scalar.copy(out=w16, in_=wt_ps)
            nc.sync.dma_start(out=w_scr[:], in_=w16)
            wb_tiles = []
            for t in range(NCH):
                wb_t = wbc.tile([128, CH], F32, tag=f"wb{t}")
                for g4 in range(4):
                    g = t * 4 + g4
                    row = w_scr[g:g + 1, :]
                    bcast = bass.AP(
                        tensor=row.tensor, offset=row.offset,
                        ap=[[0, 128]] + list(row.ap[1:]),
                    )
                    nc.sync.dma_start(
                        out=wb_t[:, g4 * 128:(g4 + 1) * 128], in_=bcast
                    )
                wb_tiles.append(wb_t)

            # ---- phase E: experts, chunk by chunk ----
            for t in range(NCH):
                tok = bass.ts(t, CH)
                xrb, xib, xnb = [], [], []
                for p in range(KD):
                    xrf = xf.tile([128, CH], F32, tag=f"xr{p}")
                    nc.sync.dma_start(out=xrf, in_=xrT[p * 128:(p + 1) * 128, tok])
                    xif = xf.tile([128, CH], F32, tag=f"xi{p}")
                    nc.sync.dma_start(out=xif, in_=xiT[p * 128:(p + 1) * 128, tok])
                    xr_b = xb.tile([128, CH], BF16, tag=f"xrb{p}")
                    nc.vector.tensor_copy(out=xr_b, in_=xrf)
                    xi_b = xb.tile([128, CH], BF16, tag=f"xib{p}")
                    nc.vector.tensor_copy(out=xi_b, in_=xif)
                    xn_b = xb.tile([128, CH], BF16, tag=f"xnb{p}")
                    nc.vector.tensor_scalar(
                        out=xn_b, in0=xif, scalar1=-1.0, scalar2=None, op0=ALU.mult
                    )
                    xrb.append(xr_b)
                    xib.append(xi_b)
                    xnb.append(xn_b)

                hrb, hib, hnb = [], [], []
                for m in range(KH):
                    msl = bass.ts(m, 128)
                    ps_hr = pp.tile([128, CH], F32, tag="hr")
                    for k in range(KD):
                        nc.tensor.matmul(
                            ps_hr, w1r_bf[k][:, msl], xrb[k],
                            start=(k == 0), stop=False,
                        )
                        nc.tensor.matmul(
                            ps_hr, w1i_bf[k][:, msl], xnb[k],
                            start=False, stop=(k == KD - 1),
                        )
                    ps_hi = pp.tile([128, CH], F32, tag="hi")
                    for k in range(KD):
                        nc.tensor.matmul(
                            ps_hi, w1i_bf[k][:, msl], xrb[k],
                            start=(k == 0), stop=False,
                        )
                        nc.tensor.matmul(
                            ps_hi, w1r_bf[k][:, msl], xib[k],
                            start=False, stop=(k == KD - 1),
                        )
                    # ComplexModReLU. First move (psum + b1) to SBUF on ACT so
                    # the PSUM banks free up fast and the PE never stalls.
                    b1r_m = b1r_sb[:, m:m + 1]
                    b1i_m = b1i_sb[:, m:m + 1]
                    mb_m = modb_sb[:, m:m + 1]
                    hrf = tmp.tile([128, CH], F32, tag="tE")
                    nc.scalar.activation(out=hrf, in_=ps_hr, func=AF.Identity, bias=b1r_m)
                    hif = tmp.tile([128, CH], F32, tag="tF")
                    nc.scalar.activation(out=hif, in_=ps_hi, func=AF.Identity, bias=b1i_m)
                    v1 = tmp.tile([128, CH], F32, tag="tA")
                    nc.scalar.activation(out=v1, in_=hrf, func=AF.Square)
                    v2 = tmp.tile([128, CH], F32, tag="tB")
                    nc.scalar.activation(out=v2, in_=hif, func=AF.Square)
                    nc.gpsimd.tensor_tensor(out=v1, in0=v1, in1=v2, op=ALU.add)
                    nc.scalar.activation(out=v1, in_=v1, func=AF.Sqrt, bias=eps_sb)
                    nc.scalar.activation(out=v2, in_=v1, func=AF.Relu, bias=mb_m)
                    q = tmp.tile([128, CH], F32, tag="tC")
                    nc.vector.reciprocal_approx_fast(out=q, in_=v1)
                    nc.vector.tensor_tensor(out=v2, in0=v2, in1=q, op=ALU.mult)
                    h_r = hp.tile([128, CH], BF16, tag=f"hr{m}")
                    nc.vector.tensor_tensor(out=h_r, in0=hrf, in1=v2, op=ALU.mult)
                    h_i = hp.tile([128, CH], BF16, tag=f"hi{m}")
                    nc.vector.tensor_tensor(out=h_i, in0=hif, in1=v2, op=ALU.mult)
                    h_n = hp.tile([128, CH], BF16, tag=f"hn{m}")
                    nc.vector.tensor_scalar(
                        out=h_n, in0=h_i, scalar1=-1.0, scalar2=None, op0=ALU.mult
                    )
                    hrb.append(h_r)
                    hib.append(h_i)
                    hnb.append(h_n)

                for m4 in range(MD):
                    msl = bass.ts(m4, 128)
                    ps_or = pp.tile([128, CH], F32, tag="or")
                    for k in range(KH):
                        nc.tensor.matmul(
                            ps_or, w2r_bf[k][:, msl], hrb[k],
                            start=(k == 0), stop=False,
                        )
                        nc.tensor.matmul(
                            ps_or, w2i_bf[k][:, msl], hnb[k],
                            start=False, stop=(k == KH - 1),
                        )
                    ps_oi = pp.tile([128, CH], F32, tag="oi")
                    for k in range(KH):
                        nc.tensor.matmul(
                            ps_oi, w2i_bf[k][:, msl], hrb[k],
                            start=(k == 0), stop=False,
                        )
                        nc.tensor.matmul(
                            ps_oi, w2r_bf[k][:, msl], hib[k],
                            start=False, stop=(k == KH - 1),
                        )
                    o_r = op.tile([128, CH], F32, tag="osr")
                    nc.vector.scalar_tensor_tensor(
                        out=o_r, in0=ps_or, scalar=b2r_sb[:, m4:m4 + 1],
                        in1=wb_tiles[t], op0=ALU.add, op1=ALU.mult,
                    )
                    nc.sync.dma_start(
                        out=out_r[m4 * 128:(m4 + 1) * 128, tok], in_=o_r
                    )
                    o_i = op.tile([128, CH], F32, tag="osi")
                    nc.vector.scalar_tensor_tensor(
                        out=o_i, in0=ps_oi, scalar=b2i_sb[:, m4:m4 + 1],
                        in1=wb_tiles[t], op0=ALU.add, op1=ALU.mult,
                    )
                    nc.sync.dma_start(
                        out=out_i[m4 * 128:(m4 + 1) * 128, tok], in_=o_i
                    )

    nc.compile()
    return nc


def kernel(**inputs):
    global LAST_RESULT
    f32 = lambda a: np.ascontiguousarray(np.asarray(a, dtype=np.float32))
    xr = f32(inputs["x_real"]).reshape(NT, D).T.copy()
    xi = f32(inputs["x_imag"]).reshape(NT, D).T.copy()
    gW = f32(inputs["gate_W"])
    gb = f32(inputs["gate_b"])
    W1r, W1i = f32(inputs["W1r"]), f32(inputs["W1i"])
    W2r, W2i = f32(inputs["W2r"]), f32(inputs["W2i"])
    b1r, b1i = f32(inputs["b1r"]), f32(inputs["b1i"])
    modb = f32(inputs["mod_b"])
    b2r, b2i = f32(inputs["b2r"]), f32(inputs["b2i"])

    if "nc" not in _CACHE:
        _CACHE["nc"] = _build_nc()
    nc = _CACHE["nc"]

    in_maps = []
    for c in range(E):
        perm = [c] + [e for e in range(E) if e != c]
        gWp = np.ascontiguousarray(
            gW[:, perm].reshape(8, 128, 8).transpose(1, 0, 2)
        )
        in_maps.append({
            "xrT": xr, "xiT": xi,
            "gWp": gWp,
            "gb": np.ascontiguousarray(gb[perm].reshape(8, 1)),
            "b1r": np.ascontiguousarray(b1r[c].reshape(KH, 128).T),
            "b1i": np.ascontiguousarray(b1i[c].reshape(KH, 128).T),
            "modb": np.ascontiguousarray(modb[c].reshape(KH, 128).T),
            "b2r": np.ascontiguousarray(b2r[c].reshape(MD, 128).T),
            "b2i": np.ascontiguousarray(b2i[c].reshape(MD, 128).T),
            "W1r": np.ascontiguousarray(W1r[c]),
            "W1i": np.ascontiguousarray(W1i[c]),
            "W2r": np.ascontiguousarray(W2r[c]),
            "W2i": np.ascontiguousarray(W2i[c]),
        })

    res = run_bass_kernel_spmd(nc, in_maps, list(range(E)))
    LAST_RESULT = res
    acc_r = np.zeros((D, NT), np.float32)
    acc_i = np.zeros((D, NT), np.float32)
    for c in range(E):
        acc_r += res.results[c]["out_r"]
        acc_i += res.results[c]["out_i"]
    out_r = np.ascontiguousarray(acc_r.T).reshape(B, S, D)
    out_i = np.ascontiguousarray(acc_i.T).reshape(B, S, D)
    return out_r, out_i


# revision 11
# speedup vs baseline: 1.2473x; 1.1575x over previous
"""ComplexMoELayer TRN2 kernel: dense expert-parallel across 8 NeuronCores.

Layout: everything on-device is [feature, token] ("option B"):
  - host feeds x^T [D=512, NT=2048] fp32 (both real/imag)
  - L1: h[m-tile] = sum_k W1[k,m].T @ xT[k]  -> PSUM [128, CH]
  - ComplexModReLU on PSUM tiles, emit bf16 h tiles for L2
  - L2: o[m4]  = sum_k W2[k,m4].T @ h[k]     -> PSUM [128, CH]
  - out = (o + b2) * w_token  (w = top1 routing weight, 0 for foreign tokens)
Host sums the 8 per-core partial outputs (disjoint support) and transposes back.

Gating runs in fp32 (routing argmax needs ~1e-4 accuracy; min top-2 gap of the
score distribution is ~2.5e-4):  amp = sqrt(xr^2+xi^2),
phase = 2*atan(xi/(amp+xr)),  scores^T = gate_W^T @ [amp;phase]^T.
Per-core gate_W columns are permuted so that "my expert" is always index 0,
keeping the program SPMD-identical across cores.
"""

import numpy as np

import concourse.bass as bass
import concourse.mybir as mybir
import concourse.tile as tile
from concourse import bacc
from concourse.bass_utils import run_bass_kernel_spmd
from concourse.masks import make_identity

F32 = mybir.dt.float32
BF16 = mybir.dt.bfloat16
AF = mybir.ActivationFunctionType
ALU = mybir.AluOpType

E, D, H = 8, 512, 2048
B, S = 4, 512
NT = B * S            # 2048 tokens
CH = 512              # tokens per chunk
NCH = NT // CH        # 4 chunks
KD = D // 128         # 4  k-tiles over D
KH = H // 128         # 16 k-tiles over H
MD = D // 128         # 4  m-tiles of output D
EPS = 1e-10

_CACHE: dict = {}
LAST_RESULT = None    # test harness reads exec_time_ns from here


def _build_nc():
    nc = bacc.Bacc("TRN2", target_bir_lowering=False, debug=False)

    xrT = nc.dram_tensor("xrT", [D, NT], F32, kind="ExternalInput")
    xiT = nc.dram_tensor("xiT", [D, NT], F32, kind="ExternalInput")
    gWp = nc.dram_tensor("gWp", [128, 8, 8], F32, kind="ExternalInput")
    gb = nc.dram_tensor("gb", [8, 1], F32, kind="ExternalInput")
    b1r_d = nc.dram_tensor("b1r", [128, KH], F32, kind="ExternalInput")
    b1i_d = nc.dram_tensor("b1i", [128, KH], F32, kind="ExternalInput")
    modb_d = nc.dram_tensor("modb", [128, KH], F32, kind="ExternalInput")
    b2r_d = nc.dram_tensor("b2r", [128, MD], F32, kind="ExternalInput")
    b2i_d = nc.dram_tensor("b2i", [128, MD], F32, kind="ExternalInput")
    W1r_d = nc.dram_tensor("W1r", [D, H], F32, kind="ExternalInput")
    W1i_d = nc.dram_tensor("W1i", [D, H], F32, kind="ExternalInput")
    W2r_d = nc.dram_tensor("W2r", [H, D], F32, kind="ExternalInput")
    W2i_d = nc.dram_tensor("W2i", [H, D], F32, kind="ExternalInput")
    out_r = nc.dram_tensor("out_r", [D, NT], F32, kind="ExternalOutput")
    out_i = nc.dram_tensor("out_i", [D, NT], F32, kind="ExternalOutput")
    w_scr = nc.dram_tensor("w_scr", [KH, 128], F32)  # internal scratch for w rows

    with tile.TileContext(nc) as tc:
        import contextlib

        ctx = contextlib.ExitStack()
        with ctx:
            smalls = ctx.enter_context(tc.tile_pool(name="smalls", bufs=1))
            wload = ctx.enter_context(tc.tile_pool(name="wload", bufs=2))
            wbf = ctx.enter_context(tc.tile_pool(name="wbf", bufs=1))
            xf = ctx.enter_context(tc.tile_pool(name="xf", bufs=2))
            xb = ctx.enter_context(tc.tile_pool(name="xb", bufs=1))
            tmp = ctx.enter_context(tc.tile_pool(name="tmp", bufs=2))
            hp = ctx.enter_context(tc.tile_pool(name="hp", bufs=1))
            op = ctx.enter_context(tc.tile_pool(name="op", bufs=2))
            wbc = ctx.enter_context(tc.tile_pool(name="wbc", bufs=1))
            scp = ctx.enter_context(tc.tile_pool(name="scp", bufs=2))
            pp = ctx.enter_context(tc.tile_pool(name="pp", bufs=2, space="PSUM"))

            # ---- small constants ----
            gw_sb = smalls.tile([128, 8, 8], F32)
            nc.sync.dma_start(out=gw_sb, in_=gWp[:])
            gb_sb = smalls.tile([8, 1], F32)
            nc.sync.dma_start(out=gb_sb, in_=gb[:])
            b1r_sb = smalls.tile([128, KH], F32)
            nc.sync.dma_start(out=b1r_sb, in_=b1r_d[:])
            b1i_sb = smalls.tile([128, KH], F32)
            nc.sync.dma_start(out=b1i_sb, in_=b1i_d[:])
            modb_sb = smalls.tile([128, KH], F32)
            nc.sync.dma_start(out=modb_sb, in_=modb_d[:])
            b2r_sb = smalls.tile([128, MD], F32)
            nc.sync.dma_start(out=b2r_sb, in_=b2r_d[:])
            b2i_sb = smalls.tile([128, MD], F32)
            nc.sync.dma_start(out=b2i_sb, in_=b2i_d[:])
            ident = smalls.tile([128, 128], F32)
            make_identity(nc, ident)
            eps_sb = smalls.tile([128, 1], F32)
            nc.vector.memset(eps_sb, EPS)
            scores_t = smalls.tile([128, KH, 8], F32)
            e_t = smalls.tile([128, KH, 8], F32)
            mx = smalls.tile([128, KH], F32)
            sm = smalls.tile([128, KH], F32)
            rs = smalls.tile([128, KH], F32)
            pe = smalls.tile([128, KH], F32)
            msk = smalls.tile([128, KH], F32)
            w_pt = smalls.tile([128, KH], F32)

            # ---- expert weights: DMA fp32 -> cast bf16 (resident) ----
            w1r_bf, w1i_bf = [], []
            for k in range(KD):
                t_r = wbf.tile([128, H], BF16, tag=f"w1r{k}")
                t_i = wbf.tile([128, H], BF16, tag=f"w1i{k}")
                for c4 in range(4):
                    sl = bass.ts(c4, 512)
                    wt = wload.tile([128, 512], F32, tag="wl")
                    nc.sync.dma_start(out=wt, in_=W1r_d[k * 128:(k + 1) * 128, sl])
                    nc.vector.tensor_copy(out=t_r[:, sl], in_=wt)
                    wt2 = wload.tile([128, 512], F32, tag="wl")
                    nc.sync.dma_start(out=wt2, in_=W1i_d[k * 128:(k + 1) * 128, sl])
                    nc.vector.tensor_copy(out=t_i[:, sl], in_=wt2)
                w1r_bf.append(t_r)
                w1i_bf.append(t_i)
            w2r_bf, w2i_bf = [], []
            for k in range(KH):
                wt = wload.tile([128, 512], F32, tag="wl")
                nc.sync.dma_start(out=wt, in_=W2r_d[k * 128:(k + 1) * 128, :])
                t_r = wbf.tile([128, D], BF16, tag=f"w2r{k}")
                nc.scalar.copy(out=t_r, in_=wt)
                wt2 = wload.tile([128, 512], F32, tag="wl")
                nc.sync.dma_start(out=wt2, in_=W2i_d[k * 128:(k + 1) * 128, :])
                t_i = wbf.tile([128, D], BF16, tag=f"w2i{k}")
                nc.scalar.copy(out=t_i, in_=wt2)
                w2r_bf.append(t_r)
                w2i_bf.append(t_i)

            # ---- pipelined per-chunk: gating -> softmax/w -> L1 -> modrelu -> L2
            for t in range(NCH):
                tok = bass.ts(t, CH)
                # x fp32 load (shared by gating and bf16 casts)
                xrf, xif, xrb, xib, xnb = [], [], [], [], []
                for p in range(KD):
                    xr = xf.tile([128, CH], F32, tag=f"xr{p}")
                    nc.sync.dma_start(out=xr, in_=xrT[p * 128:(p + 1) * 128, tok])
                    xi = xf.tile([128, CH], F32, tag=f"xi{p}")
                    nc.sync.dma_start(out=xi, in_=xiT[p * 128:(p + 1) * 128, tok])
                    xrf.append(xr)
                    xif.append(xi)
                    xr_b = xb.tile([128, CH], BF16, tag=f"xrb{p}")
                    nc.vector.tensor_copy(out=xr_b, in_=xr)
                    xi_b = xb.tile([128, CH], BF16, tag=f"xib{p}")
                    nc.vector.tensor_copy(out=xi_b, in_=xi)
                    xn_b = xb.tile([128, CH], BF16, tag=f"xnb{p}")
                    nc.vector.tensor_scalar(
                        out=xn_b, in0=xi, scalar1=-1.0, scalar2=None, op0=ALU.mult
                    )
                    xrb.append(xr_b)
                    xib.append(xi_b)
                    xnb.append(xn_b)
                # gating: scores^T for this chunk
                sc_ps = pp.tile([8, CH], F32, tag="g")
                for p in range(KD):
                    xr, xi = xrf[p], xif[p]
                    v = tmp.tile([128, CH], F32, tag="tA")
                    nc.scalar.activation(out=v, in_=xr, func=AF.Square)
                    v2 = tmp.tile([128, CH], F32, tag="tB")
                    nc.scalar.activation(out=v2, in_=xi, func=AF.Square)
                    nc.gpsimd.tensor_tensor(out=v, in0=v, in1=v2, op=ALU.add)
                    amp = tmp.tile([128, CH], F32, tag="tC")
                    nc.scalar.activation(out=amp, in_=v, func=AF.Sqrt)
                    # half-angle atan2: ph = 2*atan(xi / max(amp + xr, 1e-30));
                    # the clamp keeps the seeded reciprocal defined when
                    # amp+xr rounds to exactly 0 (xr<0, |xi|<<|xr|) -- atan
                    # then saturates to +-pi/2 and phase to +-pi as arctan2.
                    nc.gpsimd.tensor_tensor(out=v, in0=amp, in1=xr, op=ALU.add)
                    nc.vector.tensor_scalar(
                        out=v, in0=v, scalar1=1e-30, scalar2=None, op0=ALU.max
                    )
                    nc.vector.reciprocal_approx_fast(out=v2, in_=v)
                    nc.vector.tensor_tensor(out=v, in0=xi, in1=v2, op=ALU.mult)
                    nc.scalar.activation(out=v, in_=v, func=AF.Arctan)
                    ph = tmp.tile([128, CH], F32, tag="tD")
                    nc.vector.tensor_scalar(
                        out=ph, in0=v, scalar1=2.0, scalar2=None, op0=ALU.mult
                    )
                    nc.tensor.matmul(
                        sc_ps, gw_sb[:, p, :], amp, start=(p == 0), stop=False
                    )
                    nc.tensor.matmul(
                        sc_ps, gw_sb[:, KD + p, :], ph, start=False, stop=(p == KD - 1)
                    )
                sc_sb = scp.tile([8, CH], F32, tag="sc", bufs=1)
                nc.vector.tensor_scalar(
                    out=sc_sb, in0=sc_ps, scalar1=gb_sb[:, 0:1], scalar2=None,
                    op0=ALU.add,
                )
                for g4 in range(4):
                    tp_ps = pp.tile([128, 8], F32, tag="g")
                    nc.tensor.transpose(
                        tp_ps, sc_sb[:, g4 * 128:(g4 + 1) * 128], ident[0:8, 0:8]
                    )
                    nc.scalar.copy(out=scores_t[:, t * 4 + g4, :], in_=tp_ps)
                # per-chunk softmax / top-1 weight (expert 0 = ours)
                gsl = slice(t * 4, (t + 1) * 4)
                nc.scalar.activation(
                    out=e_t[:, gsl, :], in_=scores_t[:, gsl, :], func=AF.Exp
                )
                nc.vector.tensor_reduce(
                    out=mx[:, gsl], in_=scores_t[:, gsl, :],
                    axis=mybir.AxisListType.X, op=ALU.max,
                )
                nc.vector.tensor_reduce(
                    out=sm[:, gsl], in_=e_t[:, gsl, :],
                    axis=mybir.AxisListType.X, op=ALU.add,
                )
                nc.vector.reciprocal_approx_fast(out=rs[:, gsl], in_=sm[:, gsl])
                nc.vector.tensor_tensor(
                    out=pe[:, gsl], in0=e_t[:, gsl, 0], in1=rs[:, gsl], op=ALU.mult
                )
                nc.vector.tensor_tensor(
                    out=msk[:, gsl], in0=scores_t[:, gsl, 0], in1=mx[:, gsl],
                    op=ALU.is_ge,
                )
                nc.vector.tensor_tensor(
                    out=w_pt[:, gsl], in0=pe[:, gsl], in1=msk[:, gsl], op=ALU.mult
                )
                wt_ps = pp.tile([4, 128], F32, tag="g")
                nc.tensor.transpose(wt_ps, w_pt[:, gsl], ident)
                w16c = scp.tile([4, 128], F32, tag="w16")
                nc.scalar.copy(out=w16c, in_=wt_ps)
                nc.sync.dma_start(out=w_scr[gsl, :], in_=w16c)
                wb_t = wbc.tile([128, CH], F32, tag=f"wb{t}")
                for g4 in range(4):
                    g = t * 4 + g4
                    row = w_scr[g:g + 1, :]
                    bcast = bass.AP(
                        tensor=row.tensor, offset=row.offset,
                        ap=[[0, 128]] + list(row.ap[1:]),
                    )
                    nc.sync.dma_start(
                        out=wb_t[:, g4 * 128:(g4 + 1) * 128], in_=bcast
                    )

                hrb, hib, hnb = [], [], []
                for m in range(KH):
                    msl = bass.ts(m, 128)
                    ps_hr = pp.tile([128, CH], F32, tag="hr")
                    for k in range(KD):
                        nc.tensor.matmul(
                            ps_hr, w1r_bf[k][:, msl], xrb[k],
                            start=(k == 0), stop=False,
                        )
                        nc.tensor.matmul(
                            ps_hr, w1i_bf[k][:, msl], xnb[k],
                            start=False, stop=(k == KD - 1),
                        )
                    ps_hi = pp.tile([128, CH], F32, tag="hi")
                    for k in range(KD):
                        nc.tensor.matmul(
                            ps_hi, w1i_bf[k][:, msl], xrb[k],
                            start=(k == 0), stop=False,
                        )
                        nc.tensor.matmul(
                            ps_hi, w1r_bf[k][:, msl], xib[k],
                            start=False, stop=(k == KD - 1),
                        )
                    # ComplexModReLU. First move (psum + b1) to SBUF on ACT so
                    # the PSUM banks free up fast and the PE never stalls.
                    b1r_m = b1r_sb[:, m:m + 1]
                    b1i_m = b1i_sb[:, m:m + 1]
                    mb_m = modb_sb[:, m:m + 1]
                    hrf = tmp.tile([128, CH], F32, tag="tE")
                    nc.scalar.activation(out=hrf, in_=ps_hr, func=AF.Identity, bias=b1r_m)
                    hif = tmp.tile([128, CH], F32, tag="tF")
                    nc.scalar.activation(out=hif, in_=ps_hi, func=AF.Identity, bias=b1i_m)
                    v1 = tmp.tile([128, CH], F32, tag="tA")
                    nc.scalar.activation(out=v1, in_=hrf, func=AF.Square)
                    v2 = tmp.tile([128, CH], F32, tag="tB")
                    nc.scalar.activation(out=v2, in_=hif, func=AF.Square)
                    nc.gpsimd.tensor_tensor(out=v1, in0=v1, in1=v2, op=ALU.add)
                    nc.scalar.activation(out=v1, in_=v1, func=AF.Sqrt, bias=eps_sb)
                    nc.scalar.activation(out=v2, in_=v1, func=AF.Relu, bias=mb_m)
                    q = tmp.tile([128, CH], F32, tag="tC")
                    nc.vector.reciprocal_approx_fast(out=q, in_=v1)
                    nc.vector.tensor_tensor(out=v2, in0=v2, in1=q, op=ALU.mult)
                    h_r = hp.tile([128, CH], BF16, tag=f"hr{m}")
                    nc.vector.tensor_tensor(out=h_r, in0=hrf, in1=v2, op=ALU.mult)
                    h_i = hp.tile([128, CH], BF16, tag=f"hi{m}")
                    nc.vector.tensor_tensor(out=h_i, in0=hif, in1=v2, op=ALU.mult)
                    h_n = hp.tile([128, CH], BF16, tag=f"hn{m}")
                    nc.vector.tensor_scalar(
                        out=h_n, in0=h_i, scalar1=-1.0, scalar2=None, op0=ALU.mult
                    )
                    hrb.append(h_r)
                    hib.append(h_i)
                    hnb.append(h_n)

                for m4 in range(MD):
                    msl = bass.ts(m4, 128)
                    ps_or = pp.tile([128, CH], F32, tag="or", bufs=1)
                    for k in range(KH):
                        nc.tensor.matmul(
                            ps_or, w2r_bf[k][:, msl], hrb[k],
                            start=(k == 0), stop=False,
                        )
                        nc.tensor.matmul(
                            ps_or, w2i_bf[k][:, msl], hnb[k],
                            start=False, stop=(k == KH - 1),
                        )
                    ps_oi = pp.tile([128, CH], F32, tag="oi", bufs=1)
                    for k in range(KH):
                        nc.tensor.matmul(
                            ps_oi, w2i_bf[k][:, msl], hrb[k],
                            start=(k == 0), stop=False,
                        )
                        nc.tensor.matmul(
                            ps_oi, w2r_bf[k][:, msl], hib[k],
                            start=False, stop=(k == KH - 1),
                        )
                    o_r = op.tile([128, CH], F32, tag="osr")
                    nc.vector.scalar_tensor_tensor(
                        out=o_r, in0=ps_or, scalar=b2r_sb[:, m4:m4 + 1],
                        in1=wb_t, op0=ALU.add, op1=ALU.mult,
                    )
                    nc.sync.dma_start(
                        out=out_r[m4 * 128:(m4 + 1) * 128, tok], in_=o_r
                    )
                    o_i = op.tile([128, CH], F32, tag="osi")
                    nc.vector.scalar_tensor_tensor(
                        out=o_i, in0=ps_oi, scalar=b2i_sb[:, m4:m4 + 1],
                        in1=wb_t, op0=ALU.add, op1=ALU.mult,
                    )
                    nc.sync.dma_start(
                        out=out_i[m4 * 128:(m4 + 1) * 128, tok], in_=o_i
                    )

    nc.compile()
    return nc


def kernel(**inputs):
    global LAST_RESULT
    f32 = lambda a: np.ascontiguousarray(np.asarray(a, dtype=np.float32))
    xr = f32(inputs["x_real"]).reshape(NT, D).T.copy()
    xi = f32(inputs["x_imag"]).reshape(NT, D).T.copy()
    gW = f32(inputs["gate_W"])
    gb = f32(inputs["gate_b"])
    W1r, W1i = f32(inputs["W1r"]), f32(inputs["W1i"])
    W2r, W2i = f32(inputs["W2r"]), f32(inputs["W2i"])
    b1r, b1i = f32(inputs["b1r"]), f32(inputs["b1i"])
    modb = f32(inputs["mod_b"])
    b2r, b2i = f32(inputs["b2r"]), f32(inputs["b2i"])

    if "nc" not in _CACHE:
        _CACHE["nc"] = _build_nc()
    nc = _CACHE["nc"]

    in_maps = []
    for c in range(E):
        perm = [c] + [e for e in range(E) if e != c]
        gWp = np.ascontiguousarray(
            gW[:, perm].reshape(8, 128, 8).transpose(1, 0, 2)
        )
        in_maps.append({
            "xrT": xr, "xiT": xi,
            "gWp": gWp,
            "gb": np.ascontiguousarray(gb[perm].reshape(8, 1)),
            "b1r": np.ascontiguousarray(b1r[c].reshape(KH, 128).T),
            "b1i": np.ascontiguousarray(b1i[c].reshape(KH, 128).T),
            "modb": np.ascontiguousarray(modb[c].reshape(KH, 128).T),
            "b2r": np.ascontiguousarray(b2r[c].reshape(MD, 128).T),
            "b2i": np.ascontiguousarray(b2i[c].reshape(MD, 128).T),
            "W1r": np.ascontiguousarray(W1r[c]),
            "W1i": np.ascontiguousarray(W1i[c]),
            "W2r": np.ascontiguousarray(W2r[c]),
            "W2i": np.ascontiguousarray(W2i[c]),
        })

    res = run_bass_kernel_spmd(nc, in_maps, list(range(E)))
    LAST_RESULT = res
    acc_r = np.zeros((D, NT), np.float32)
    acc_i = np.zeros((D, NT), np.float32)
    for c in range(E):
        acc_r += res.results[c]["out_r"]
        acc_i += res.results[c]["out_i"]
    out_r = np.ascontiguousarray(acc_r.T).reshape(B, S, D)
    out_i = np.ascontiguousarray(acc_i.T).reshape(B, S, D)
    return out_r, out_i


# revision 12
# speedup vs baseline: 1.2597x; 1.0099x over previous
"""ComplexMoELayer TRN2 kernel: dense expert-parallel across 8 NeuronCores.

Layout: everything on-device is [feature, token] ("option B"):
  - host feeds x^T [D=512, NT=2048] fp32 (both real/imag)
  - L1: h[m-tile] = sum_k W1[k,m].T @ xT[k]  -> PSUM [128, CH]
  - ComplexModReLU on PSUM tiles, emit bf16 h tiles for L2
  - L2: o[m4]  = sum_k W2[k,m4].T @ h[k]     -> PSUM [128, CH]
  - out = (o + b2) * w_token  (w = top1 routing weight, 0 for foreign tokens)
Host sums the 8 per-core partial outputs (disjoint support) and transposes back.

Gating runs in fp32 (routing argmax needs ~1e-4 accuracy; min top-2 gap of the
score distribution is ~2.5e-4):  amp = sqrt(xr^2+xi^2),
phase = 2*atan(xi/(amp+xr)),  scores^T = gate_W^T @ [amp;phase]^T.
Per-core gate_W columns are permuted so that "my expert" is always index 0,
keeping the program SPMD-identical across cores.
"""

import numpy as np

import concourse.bass as bass
import concourse.mybir as mybir
import concourse.tile as tile
from concourse import bacc
from concourse.bass_utils import run_bass_kernel_spmd
from concourse.masks import make_identity

F32 = mybir.dt.float32
BF16 = mybir.dt.bfloat16
AF = mybir.ActivationFunctionType
ALU = mybir.AluOpType

E, D, H = 8, 512, 2048
B, S = 4, 512
NT = B * S            # 2048 tokens
CH = 512              # tokens per chunk
NCH = NT // CH        # 4 chunks
KD = D // 128         # 4  k-tiles over D
KH = H // 128         # 16 k-tiles over H
MD = D // 128         # 4  m-tiles of output D
EPS = 1e-10

_CACHE: dict = {}
LAST_RESULT = None    # test harness reads exec_time_ns from here


def _build_nc():
    nc = bacc.Bacc("TRN2", target_bir_lowering=False, debug=False)

    xrT = nc.dram_tensor("xrT", [D, NT], F32, kind="ExternalInput")
    xiT = nc.dram_tensor("xiT", [D, NT], F32, kind="ExternalInput")
    gWp = nc.dram_tensor("gWp", [128, 8, 8], F32, kind="ExternalInput")
    gb = nc.dram_tensor("gb", [8, 1], F32, kind="ExternalInput")
    b1r_d = nc.dram_tensor("b1r", [128, KH], F32, kind="ExternalInput")
    b1i_d = nc.dram_tensor("b1i", [128, KH], F32, kind="ExternalInput")
    modb_d = nc.dram_tensor("modb", [128, KH], F32, kind="ExternalInput")
    b2r_d = nc.dram_tensor("b2r", [128, MD], F32, kind="ExternalInput")
    b2i_d = nc.dram_tensor("b2i", [128, MD], F32, kind="ExternalInput")
    W1r_d = nc.dram_tensor("W1r", [D, H], F32, kind="ExternalInput")
    W1i_d = nc.dram_tensor("W1i", [D, H], F32, kind="ExternalInput")
    W2r_d = nc.dram_tensor("W2r", [H, D], F32, kind="ExternalInput")
    W2i_d = nc.dram_tensor("W2i", [H, D], F32, kind="ExternalInput")
    out_r = nc.dram_tensor("out_r", [D, NT], F32, kind="ExternalOutput")
    out_i = nc.dram_tensor("out_i", [D, NT], F32, kind="ExternalOutput")
    w_scr = nc.dram_tensor("w_scr", [KH, 128], F32)  # internal scratch for w rows

    with tile.TileContext(nc) as tc:
        import contextlib

        ctx = contextlib.ExitStack()
        with ctx:
            smalls = ctx.enter_context(tc.tile_pool(name="smalls", bufs=1))
            wload = ctx.enter_context(tc.tile_pool(name="wload", bufs=2))
            wbf = ctx.enter_context(tc.tile_pool(name="wbf", bufs=1))
            xf = ctx.enter_context(tc.tile_pool(name="xf", bufs=2))
            xb = ctx.enter_context(tc.tile_pool(name="xb", bufs=1))
            tmp = ctx.enter_context(tc.tile_pool(name="tmp", bufs=2))
            hp = ctx.enter_context(tc.tile_pool(name="hp", bufs=1))
            op = ctx.enter_context(tc.tile_pool(name="op", bufs=2))
            wbc = ctx.enter_context(tc.tile_pool(name="wbc", bufs=1))
            scp = ctx.enter_context(tc.tile_pool(name="scp", bufs=2))
            pp = ctx.enter_context(tc.tile_pool(name="pp", bufs=2, space="PSUM"))

            # ---- small constants ----
            gw_sb = smalls.tile([128, 8, 8], F32)
            nc.sync.dma_start(out=gw_sb, in_=gWp[:])
            gb_sb = smalls.tile([8, 1], F32)
            nc.sync.dma_start(out=gb_sb, in_=gb[:])
            b1r_sb = smalls.tile([128, KH], F32)
            nc.sync.dma_start(out=b1r_sb, in_=b1r_d[:])
            b1i_sb = smalls.tile([128, KH], F32)
            nc.sync.dma_start(out=b1i_sb, in_=b1i_d[:])
            modb_sb = smalls.tile([128, KH], F32)
            nc.sync.dma_start(out=modb_sb, in_=modb_d[:])
            b2r_sb = smalls.tile([128, MD], F32)
            nc.sync.dma_start(out=b2r_sb, in_=b2r_d[:])
            b2i_sb = smalls.tile([128, MD], F32)
            nc.sync.dma_start(out=b2i_sb, in_=b2i_d[:])
            ident = smalls.tile([128, 128], F32)
            make_identity(nc, ident)
            eps_sb = smalls.tile([128, 1], F32)
            nc.vector.memset(eps_sb, EPS)
            scores_t = smalls.tile([128, KH, 8], F32)
            e_t = smalls.tile([128, KH, 8], F32)
            mx = smalls.tile([128, KH], F32)
            sm = smalls.tile([128, KH], F32)
            rs = smalls.tile([128, KH], F32)
            pe = smalls.tile([128, KH], F32)
            msk = smalls.tile([128, KH], F32)
            w_pt = smalls.tile([128, KH], F32)

            # ---- expert weights: DMA fp32 -> cast bf16 (resident) ----
            w1r_bf, w1i_bf = [], []
            for k in range(KD):
                t_r = wbf.tile([128, H], BF16, tag=f"w1r{k}")
                t_i = wbf.tile([128, H], BF16, tag=f"w1i{k}")
                for c4 in range(4):
                    sl = bass.ts(c4, 512)
                    wt = wload.tile([128, 512], F32, tag="wl")
                    nc.sync.dma_start(out=wt, in_=W1r_d[k * 128:(k + 1) * 128, sl])
                    nc.vector.tensor_copy(out=t_r[:, sl], in_=wt)
                    wt2 = wload.tile([128, 512], F32, tag="wl")
                    nc.sync.dma_start(out=wt2, in_=W1i_d[k * 128:(k + 1) * 128, sl])
                    nc.vector.tensor_copy(out=t_i[:, sl], in_=wt2)
                w1r_bf.append(t_r)
                w1i_bf.append(t_i)
            w2r_bf, w2i_bf = [], []
            for k in range(KH):
                wt = wload.tile([128, 512], F32, tag="wl")
                nc.sync.dma_start(out=wt, in_=W2r_d[k * 128:(k + 1) * 128, :])
                t_r = wbf.tile([128, D], BF16, tag=f"w2r{k}")
                nc.scalar.copy(out=t_r, in_=wt)
                wt2 = wload.tile([128, 512], F32, tag="wl")
                nc.sync.dma_start(out=wt2, in_=W2i_d[k * 128:(k + 1) * 128, :])
                t_i = wbf.tile([128, D], BF16, tag=f"w2i{k}")
                nc.scalar.copy(out=t_i, in_=wt2)
                w2r_bf.append(t_r)
                w2i_bf.append(t_i)

            # ---- software-pipelined chunks: gating(t) overlaps experts(t-1)
            def emit_gating(t):
                tok = bass.ts(t, CH)
                xrf, xif = [], []
                for p in range(KD):
                    xr = xf.tile([128, CH], F32, tag=f"xr{p}", name=f"xr_{t}_{p}")
                    nc.sync.dma_start(out=xr, in_=xrT[p * 128:(p + 1) * 128, tok])
                    xi = xf.tile([128, CH], F32, tag=f"xi{p}", name=f"xi_{t}_{p}")
                    nc.sync.dma_start(out=xi, in_=xiT[p * 128:(p + 1) * 128, tok])
                    xrf.append(xr)
                    xif.append(xi)
                sc_ps = pp.tile([8, CH], F32, tag="g", name=f"scps_{t}")
                for p in range(KD):
                    xr, xi = xrf[p], xif[p]
                    v = tmp.tile([128, CH], F32, tag="tA", name=f"gv_{t}_{p}")
                    nc.scalar.activation(out=v, in_=xr, func=AF.Square)
                    v2 = tmp.tile([128, CH], F32, tag="tB", name=f"gv2_{t}_{p}")
                    nc.scalar.activation(out=v2, in_=xi, func=AF.Square)
                    nc.gpsimd.tensor_tensor(out=v, in0=v, in1=v2, op=ALU.add)
                    amp = tmp.tile([128, CH], F32, tag="tC", name=f"gamp_{t}_{p}")
                    nc.scalar.activation(out=amp, in_=v, func=AF.Sqrt)
                    # half-angle atan2: ph = 2*atan(xi / max(amp + xr, 1e-30));
                    # the clamp keeps the seeded reciprocal defined when amp+xr
                    # rounds to exactly 0 (xr<0, |xi|<<|xr|) -- atan then
                    # saturates to +-pi/2 and phase to +-pi as arctan2 does.
                    nc.gpsimd.tensor_tensor(out=v, in0=amp, in1=xr, op=ALU.add)
                    nc.vector.tensor_scalar(
                        out=v, in0=v, scalar1=1e-30, scalar2=None, op0=ALU.max
                    )
                    nc.vector.reciprocal_approx_fast(out=v2, in_=v)
                    nc.vector.tensor_tensor(out=v, in0=xi, in1=v2, op=ALU.mult)
                    nc.scalar.activation(out=v, in_=v, func=AF.Arctan)
                    ph = tmp.tile([128, CH], F32, tag="tD", name=f"gph_{t}_{p}")
                    nc.vector.tensor_scalar(
                        out=ph, in0=v, scalar1=2.0, scalar2=None, op0=ALU.mult
                    )
                    nc.tensor.matmul(
                        sc_ps, gw_sb[:, p, :], amp, start=(p == 0), stop=False
                    )
                    nc.tensor.matmul(
                        sc_ps, gw_sb[:, KD + p, :], ph, start=False, stop=(p == KD - 1)
                    )
                sc_sb = scp.tile([8, CH], F32, tag="sc", bufs=1, name=f"scsb_{t}")
                nc.vector.tensor_scalar(
                    out=sc_sb, in0=sc_ps, scalar1=gb_sb[:, 0:1], scalar2=None,
                    op0=ALU.add,
                )
                for g4 in range(4):
                    tp_ps = pp.tile([128, 8], F32, tag="g", name=f"tpps_{t}_{g4}")
                    nc.tensor.transpose(
                        tp_ps, sc_sb[:, g4 * 128:(g4 + 1) * 128], ident[0:8, 0:8]
                    )
                    nc.scalar.copy(out=scores_t[:, t * 4 + g4, :], in_=tp_ps)
                # per-chunk softmax / top-1 weight (expert 0 = ours)
                gsl = slice(t * 4, (t + 1) * 4)
                nc.scalar.activation(
                    out=e_t[:, gsl, :], in_=scores_t[:, gsl, :], func=AF.Exp
                )
                nc.vector.tensor_reduce(
                    out=mx[:, gsl], in_=scores_t[:, gsl, :],
                    axis=mybir.AxisListType.X, op=ALU.max,
                )
                nc.vector.tensor_reduce(
                    out=sm[:, gsl], in_=e_t[:, gsl, :],
                    axis=mybir.AxisListType.X, op=ALU.add,
                )
                nc.vector.reciprocal_approx_fast(out=rs[:, gsl], in_=sm[:, gsl])
                nc.vector.tensor_tensor(
                    out=pe[:, gsl], in0=e_t[:, gsl, 0], in1=rs[:, gsl], op=ALU.mult
                )
                nc.vector.tensor_tensor(
                    out=msk[:, gsl], in0=scores_t[:, gsl, 0], in1=mx[:, gsl],
                    op=ALU.is_ge,
                )
                nc.vector.tensor_tensor(
                    out=w_pt[:, gsl], in0=pe[:, gsl], in1=msk[:, gsl], op=ALU.mult
                )
                wt_ps = pp.tile([4, 128], F32, tag="g", name=f"wtps_{t}")
                nc.tensor.transpose(wt_ps, w_pt[:, gsl], ident)
                w16c = scp.tile([4, 128], F32, tag="w16", name=f"w16c_{t}")
                nc.scalar.copy(out=w16c, in_=wt_ps)
                nc.sync.dma_start(out=w_scr[gsl, :], in_=w16c)
                wb_t = wbc.tile([128, CH], F32, tag=f"wb{t}", name=f"wb_{t}")
                for g4 in range(4):
                    g = t * 4 + g4
                    row = w_scr[g:g + 1, :]
                    bcast = bass.AP(
                        tensor=row.tensor, offset=row.offset,
                        ap=[[0, 128]] + list(row.ap[1:]),
                    )
                    nc.sync.dma_start(
                        out=wb_t[:, g4 * 128:(g4 + 1) * 128], in_=bcast
                    )
                return {"xrf": xrf, "xif": xif, "wb": wb_t, "tok": tok}

            def emit_experts(t, st):
                tok = st["tok"]
                wb_t = st["wb"]
                xrb, xib, xnb = [], [], []
                for p in range(KD):
                    xr_b = xb.tile([128, CH], BF16, tag=f"xrb{p}", name=f"xrb_{t}_{p}")
                    nc.vector.tensor_copy(out=xr_b, in_=st["xrf"][p])
                    xi_b = xb.tile([128, CH], BF16, tag=f"xib{p}", name=f"xib_{t}_{p}")
                    nc.vector.tensor_copy(out=xi_b, in_=st["xif"][p])
                    xn_b = xb.tile([128, CH], BF16, tag=f"xnb{p}", name=f"xnb_{t}_{p}")
                    nc.vector.tensor_scalar(
                        out=xn_b, in0=st["xif"][p], scalar1=-1.0, scalar2=None,
                        op0=ALU.mult,
                    )
                    xrb.append(xr_b)
                    xib.append(xi_b)
                    xnb.append(xn_b)

                hrb, hib, hnb = [], [], []
                for m in range(KH):
                    msl = bass.ts(m, 128)
                    ps_hr = pp.tile([128, CH], F32, tag="hr", name=f"pshr_{t}_{m}")
                    for k in range(KD):
                        nc.tensor.matmul(
                            ps_hr, w1r_bf[k][:, msl], xrb[k],
                            start=(k == 0), stop=False,
                        )
                        nc.tensor.matmul(
                            ps_hr, w1i_bf[k][:, msl], xnb[k],
                            start=False, stop=(k == KD - 1),
                        )
                    ps_hi = pp.tile([128, CH], F32, tag="hi", name=f"pshi_{t}_{m}")
                    for k in range(KD):
                        nc.tensor.matmul(
                            ps_hi, w1i_bf[k][:, msl], xrb[k],
                            start=(k == 0), stop=False,
                        )
                        nc.tensor.matmul(
                            ps_hi, w1r_bf[k][:, msl], xib[k],
                            start=False, stop=(k == KD - 1),
                        )
                    # ComplexModReLU. Move (psum + b1) to SBUF on ACT first so
                    # the PSUM banks free fast and the PE never stalls.
                    b1r_m = b1r_sb[:, m:m + 1]
                    b1i_m = b1i_sb[:, m:m + 1]
                    mb_m = modb_sb[:, m:m + 1]
                    hrf = tmp.tile([128, CH], F32, tag="tE", name=f"hrf_{t}_{m}")
                    nc.scalar.activation(
                        out=hrf, in_=ps_hr, func=AF.Identity, bias=b1r_m
                    )
                    hif = tmp.tile([128, CH], F32, tag="tF", name=f"hif_{t}_{m}")
                    nc.scalar.activation(
                        out=hif, in_=ps_hi, func=AF.Identity, bias=b1i_m
                    )
                    v1 = tmp.tile([128, CH], F32, tag="tA", name=f"mv1_{t}_{m}")
                    nc.scalar.activation(out=v1, in_=hrf, func=AF.Square)
                    v2 = tmp.tile([128, CH], F32, tag="tB", name=f"mv2_{t}_{m}")
                    nc.scalar.activation(out=v2, in_=hif, func=AF.Square)
                    nc.gpsimd.tensor_tensor(out=v1, in0=v1, in1=v2, op=ALU.add)
                    nc.scalar.activation(out=v1, in_=v1, func=AF.Sqrt, bias=eps_sb)
                    nc.scalar.activation(out=v2, in_=v1, func=AF.Relu, bias=mb_m)
                    q = tmp.tile([128, CH], F32, tag="tC", name=f"mq_{t}_{m}")
                    nc.vector.reciprocal_approx_fast(out=q, in_=v1)
                    nc.vector.tensor_tensor(out=v2, in0=v2, in1=q, op=ALU.mult)
                    h_r = hp.tile([128, CH], BF16, tag=f"hr{m}", name=f"hr_{t}_{m}")
                    nc.vector.tensor_tensor(out=h_r, in0=hrf, in1=v2, op=ALU.mult)
                    h_i = hp.tile([128, CH], BF16, tag=f"hi{m}", name=f"hi_{t}_{m}")
                    nc.vector.tensor_tensor(out=h_i, in0=hif, in1=v2, op=ALU.mult)
                    h_n = hp.tile([128, CH], BF16, tag=f"hn{m}", name=f"hn_{t}_{m}")
                    nc.vector.tensor_scalar(
                        out=h_n, in0=h_i, scalar1=-1.0, scalar2=None, op0=ALU.mult
                    )
                    hrb.append(h_r)
                    hib.append(h_i)
                    hnb.append(h_n)

                for m4 in range(MD):
                    msl = bass.ts(m4, 128)
                    ps_or = pp.tile([128, CH], F32, tag="or", bufs=1, name=f"psor_{t}_{m4}")
                    for k in range(KH):
                        nc.tensor.matmul(
                            ps_or, w2r_bf[k][:, msl], hrb[k],
                            start=(k == 0), stop=False,
                        )
                        nc.tensor.matmul(
                            ps_or, w2i_bf[k][:, msl], hnb[k],
                            start=False, stop=(k == KH - 1),
                        )
                    ps_oi = pp.tile([128, CH], F32, tag="oi", bufs=1, name=f"psoi_{t}_{m4}")
                    for k in range(KH):
                        nc.tensor.matmul(
                            ps_oi, w2i_bf[k][:, msl], hrb[k],
                            start=(k == 0), stop=False,
                        )
                        nc.tensor.matmul(
                            ps_oi, w2r_bf[k][:, msl], hib[k],
                            start=False, stop=(k == KH - 1),
                        )
                    o_r = op.tile([128, CH], F32, tag="osr", name=f"or_{t}_{m4}")
                    nc.vector.scalar_tensor_tensor(
                        out=o_r, in0=ps_or, scalar=b2r_sb[:, m4:m4 + 1],
                        in1=wb_t, op0=ALU.add, op1=ALU.mult,
                    )
                    nc.sync.dma_start(
                        out=out_r[m4 * 128:(m4 + 1) * 128, tok], in_=o_r
                    )
                    o_i = op.tile([128, CH], F32, tag="osi", name=f"oi_{t}_{m4}")
                    nc.vector.scalar_tensor_tensor(
                        out=o_i, in0=ps_oi, scalar=b2i_sb[:, m4:m4 + 1],
                        in1=wb_t, op0=ALU.add, op1=ALU.mult,
                    )
                    nc.sync.dma_start(
                        out=out_i[m4 * 128:(m4 + 1) * 128, tok], in_=o_i
                    )

            states = {}
            for t in range(NCH + 1):
                if t < NCH:
                    states[t] = emit_gating(t)
                if t >= 1:
                    emit_experts(t - 1, states.pop(t - 1))

    nc.compile()
    return nc


def kernel(**inputs):
    global LAST_RESULT
    f32 = lambda a: np.ascontiguousarray(np.asarray(a, dtype=np.float32))
    xr = f32(inputs["x_real"]).reshape(NT, D).T.copy()
    xi = f32(inputs["x_imag"]).reshape(NT, D).T.copy()
    gW = f32(inputs["gate_W"])
    gb = f32(inputs["gate_b"])
    W1r, W1i = f32(inputs["W1r"]), f32(inputs["W1i"])
    W2r, W2i = f32(inputs["W2r"]), f32(inputs["W2i"])
    b1r, b1i = f32(inputs["b1r"]), f32(inputs["b1i"])
    modb = f32(inputs["mod_b"])
    b2r, b2i = f32(inputs["b2r"]), f32(inputs["b2i"])

    if "nc" not in _CACHE:
        _CACHE["nc"] = _build_nc()
    nc = _CACHE["nc"]

    in_maps = []
    for c in range(E):
        perm = [c] + [e for e in range(E) if e != c]
        gWp = np.ascontiguousarray(
            gW[:, perm].reshape(8, 128, 8).transpose(1, 0, 2)
        )
        in_maps.append({
            "xrT": xr, "xiT": xi,
            "gWp": gWp,
            "gb": np.ascontiguousarray(gb[perm].reshape(8, 1)),
            "b1r": np.ascontiguousarray(b1r[c].reshape(KH, 128).T),
            "b1i": np.ascontiguousarray(b1i[c].reshape(KH, 128).T),
            "modb": np.ascontiguousarray(modb[c].reshape(KH, 128).T),
            "b2r": np.ascontiguousarray(b2r[c].reshape(MD, 128).T),
            "b2i": np.ascontiguousarray(b2i[c].reshape(MD, 128).T),
            "W1r": np.ascontiguousarray(W1r[c]),
            "W1i": np.ascontiguousarray(W1i[c]),
            "W2r": np.ascontiguousarray(W2r[c]),
            "W2i": np.ascontiguousarray(W2i[c]),
        })

    res = run_bass_kernel_spmd(nc, in_maps, list(range(E)))
    LAST_RESULT = res
    acc_r = np.zeros((D, NT), np.float32)
    acc_i = np.zeros((D, NT), np.float32)
    for c in range(E):
        acc_r += res.results[c]["out_r"]
        acc_i += res.results[c]["out_i"]
    out_r = np.ascontiguousarray(acc_r.T).reshape(B, S, D)
    out_i = np.ascontiguousarray(acc_i.T).reshape(B, S, D)
    return out_r, out_i


# revision 15
# speedup vs baseline: 1.2801x; 1.0163x over previous
"""ComplexMoELayer TRN2 kernel: dense expert-parallel across 8 NeuronCores.

Layout: everything on-device is [feature, token] ("option B"):
  - host feeds x^T [D=512, NT=2048] fp32 (both real/imag)
  - L1: h[m-tile] = sum_k W1[k,m].T @ xT[k]  -> PSUM [128, CH]
  - ComplexModReLU on PSUM tiles, emit bf16 h tiles for L2
  - L2: o[m4]  = sum_k W2[k,m4].T @ h[k]     -> PSUM [128, CH]
  - out = (o + b2) * w_token  (w = top1 routing weight, 0 for foreign tokens)
Host sums the 8 per-core partial outputs (disjoint support) and transposes back.

Gating runs in fp32 (routing argmax needs ~1e-4 accuracy; min top-2 gap of the
score distribution is ~2.5e-4):  amp = sqrt(xr^2+xi^2),
phase = 2*atan(xi/(amp+xr)),  scores^T = gate_W^T @ [amp;phase]^T.
Per-core gate_W columns are permuted so that "my expert" is always index 0,
keeping the program SPMD-identical across cores.
"""

import numpy as np

import concourse.bass as bass
import concourse.mybir as mybir
import concourse.tile as tile
from concourse import bacc
from concourse.bass_utils import run_bass_kernel_spmd
from concourse.masks import make_identity

F32 = mybir.dt.float32
BF16 = mybir.dt.bfloat16
AF = mybir.ActivationFunctionType
ALU = mybir.AluOpType

E, D, H = 8, 512, 2048
B, S = 4, 512
NT = B * S            # 2048 tokens
CH = 512              # tokens per chunk
NCH = NT // CH        # 4 chunks
KD = D // 128         # 4  k-tiles over D
KH = H // 128         # 16 k-tiles over H
MD = D // 128         # 4  m-tiles of output D
EPS = 1e-10

_CACHE: dict = {}
LAST_RESULT = None    # test harness reads exec_time_ns from here


def _build_nc():
    nc = bacc.Bacc("TRN2", target_bir_lowering=False, debug=False)

    xrT = nc.dram_tensor("xrT", [D, NT], F32, kind="ExternalInput")
    xiT = nc.dram_tensor("xiT", [D, NT], F32, kind="ExternalInput")
    gWp = nc.dram_tensor("gWp", [128, 8, 8], F32, kind="ExternalInput")
    gb = nc.dram_tensor("gb", [8, 1], F32, kind="ExternalInput")
    b1r_d = nc.dram_tensor("b1r", [128, KH], F32, kind="ExternalInput")
    b1i_d = nc.dram_tensor("b1i", [128, KH], F32, kind="ExternalInput")
    modb_d = nc.dram_tensor("modb", [128, KH], F32, kind="ExternalInput")
    b2r_d = nc.dram_tensor("b2r", [128, MD], F32, kind="ExternalInput")
    b2i_d = nc.dram_tensor("b2i", [128, MD], F32, kind="ExternalInput")
    W1r_d = nc.dram_tensor("W1r", [D, H], F32, kind="ExternalInput")
    W1i_d = nc.dram_tensor("W1i", [D, H], F32, kind="ExternalInput")
    W2r_d = nc.dram_tensor("W2r", [H, D], F32, kind="ExternalInput")
    W2i_d = nc.dram_tensor("W2i", [H, D], F32, kind="ExternalInput")
    out_r = nc.dram_tensor("out_r", [D, NT], F32, kind="ExternalOutput")
    out_i = nc.dram_tensor("out_i", [D, NT], F32, kind="ExternalOutput")
    w_scr = nc.dram_tensor("w_scr", [KH, 128], F32)  # internal scratch for w rows

    with tile.TileContext(nc) as tc:
        import contextlib

        ctx = contextlib.ExitStack()
        with ctx:
            smalls = ctx.enter_context(tc.tile_pool(name="smalls", bufs=1))
            wload = ctx.enter_context(tc.tile_pool(name="wload", bufs=2))  # wl tag sized below
            wbf = ctx.enter_context(tc.tile_pool(name="wbf", bufs=1))
            xf = ctx.enter_context(tc.tile_pool(name="xf", bufs=2))
            xb = ctx.enter_context(tc.tile_pool(name="xb", bufs=1))
            tmp = ctx.enter_context(tc.tile_pool(name="tmp", bufs=2))
            hp = ctx.enter_context(tc.tile_pool(name="hp", bufs=1))
            op = ctx.enter_context(tc.tile_pool(name="op", bufs=2))
            wbc = ctx.enter_context(tc.tile_pool(name="wbc", bufs=1))
            scp = ctx.enter_context(tc.tile_pool(name="scp", bufs=2))
            pp = ctx.enter_context(tc.tile_pool(name="pp", bufs=2, space="PSUM"))

            # ---- small constants ----
            gw_sb = smalls.tile([128, 8, 8], F32)
            nc.sync.dma_start(out=gw_sb, in_=gWp[:])
            gb_sb = smalls.tile([8, 1], F32)
            nc.sync.dma_start(out=gb_sb, in_=gb[:])
            b1r_sb = smalls.tile([128, KH], F32)
            nc.sync.dma_start(out=b1r_sb, in_=b1r_d[:])
            b1i_sb = smalls.tile([128, KH], F32)
            nc.sync.dma_start(out=b1i_sb, in_=b1i_d[:])
            modb_sb = smalls.tile([128, KH], F32)
            nc.sync.dma_start(out=modb_sb, in_=modb_d[:])
            b2r_sb = smalls.tile([128, MD], F32)
            nc.sync.dma_start(out=b2r_sb, in_=b2r_d[:])
            b2i_sb = smalls.tile([128, MD], F32)
            nc.sync.dma_start(out=b2i_sb, in_=b2i_d[:])
            ident = smalls.tile([128, 128], F32)
            make_identity(nc, ident)
            eps_sb = smalls.tile([128, 1], F32)
            nc.vector.memset(eps_sb, EPS)
            scores_t = smalls.tile([128, KH, 8], F32)
            e_t = smalls.tile([128, KH, 8], F32)
            mx = smalls.tile([128, KH], F32)
            sm = smalls.tile([128, KH], F32)
            rs = smalls.tile([128, KH], F32)
            pe = smalls.tile([128, KH], F32)
            msk = smalls.tile([128, KH], F32)
            w_pt = smalls.tile([128, KH], F32)

            # ---- expert weights: DMA fp32 (gpsimd queue) -> cast bf16 ----
            w1r_bf, w1i_bf = [], []
            for k in range(KD):
                t_r = wbf.tile([128, H], BF16, tag=f"w1r{k}")
                t_i = wbf.tile([128, H], BF16, tag=f"w1i{k}")
                for c2 in range(2):
                    sl = bass.ts(c2, 1024)
                    wt = wload.tile([128, 1024], F32, tag="wl", bufs=1)
                    nc.gpsimd.dma_start(out=wt, in_=W1r_d[k * 128:(k + 1) * 128, sl])
                    nc.vector.tensor_copy(out=t_r[:, sl], in_=wt)
                    wt2 = wload.tile([128, 1024], F32, tag="wl", bufs=1)
                    nc.gpsimd.dma_start(out=wt2, in_=W1i_d[k * 128:(k + 1) * 128, sl])
                    nc.vector.tensor_copy(out=t_i[:, sl], in_=wt2)
                w1r_bf.append(t_r)
                w1i_bf.append(t_i)
            # W2 packed: group g holds k-tiles g*4..g*4+3 as [128, 4, 512];
            # DRAM rows (j*128+p) -> SBUF [p, j, :]
            w2r_g, w2i_g = [], []
            W2r_r = W2r_d[:].rearrange("(g j p) d -> g p j d", g=4, j=4)
            W2i_r = W2i_d[:].rearrange("(g j p) d -> g p j d", g=4, j=4)
            for g in range(4):
                t_r = wbf.tile([128, 4, 512], BF16, tag=f"w2r{g}")
                t_i = wbf.tile([128, 4, 512], BF16, tag=f"w2i{g}")
                for c2 in range(2):
                    wt = wload.tile([128, 2, 512], F32, tag="wl", bufs=1)
                    nc.gpsimd.dma_start(out=wt, in_=W2r_r[g, :, c2 * 2:(c2 + 1) * 2, :])
                    nc.scalar.copy(out=t_r[:, c2 * 2:(c2 + 1) * 2, :], in_=wt)
                    wt2 = wload.tile([128, 2, 512], F32, tag="wl", bufs=1)
                    nc.gpsimd.dma_start(out=wt2, in_=W2i_r[g, :, c2 * 2:(c2 + 1) * 2, :])
                    nc.scalar.copy(out=t_i[:, c2 * 2:(c2 + 1) * 2, :], in_=wt2)
                w2r_g.append(t_r)
                w2i_g.append(t_i)
            w2r_bf = [w2r_g[k // 4][:, k % 4, :] for k in range(KH)]
            w2i_bf = [w2i_g[k // 4][:, k % 4, :] for k in range(KH)]

            # ---- software-pipelined chunks: gating(t) overlaps experts(t-1)
            def emit_gating(t):
                tok = bass.ts(t, CH)
                xr_pk = xf.tile([128, 4, CH], F32, tag="xr", name=f"xr_{t}")
                nc.sync.dma_start(
                    out=xr_pk, in_=xrT[:].rearrange("(q p) n -> p q n", p=128)[:, :, tok]
                )
                xi_pk = xf.tile([128, 4, CH], F32, tag="xi", name=f"xi_{t}")
                nc.sync.dma_start(
                    out=xi_pk, in_=xiT[:].rearrange("(q p) n -> p q n", p=128)[:, :, tok]
                )
                xrf = [xr_pk[:, p, :] for p in range(KD)]
                xif = [xi_pk[:, p, :] for p in range(KD)]
                sc_ps = pp.tile([8, CH], F32, tag="g", name=f"scps_{t}")
                for p in range(KD):
                    xr, xi = xrf[p], xif[p]
                    v = tmp.tile([128, CH], F32, tag="tA", name=f"gv_{t}_{p}")
                    nc.scalar.activation(out=v, in_=xr, func=AF.Square)
                    v2 = tmp.tile([128, CH], F32, tag="tB", name=f"gv2_{t}_{p}")
                    nc.scalar.activation(out=v2, in_=xi, func=AF.Square)
                    nc.gpsimd.tensor_tensor(out=v, in0=v, in1=v2, op=ALU.add)
                    amp = tmp.tile([128, CH], F32, tag="tC", name=f"gamp_{t}_{p}")
                    nc.scalar.activation(out=amp, in_=v, func=AF.Sqrt)
                    # half-angle atan2: ph = 2*atan(xi / max(amp + xr, 1e-30));
                    # the clamp keeps the seeded reciprocal defined when amp+xr
                    # rounds to exactly 0 (xr<0, |xi|<<|xr|) -- atan then
                    # saturates to +-pi/2 and phase to +-pi as arctan2 does.
                    nc.gpsimd.tensor_tensor(out=v, in0=amp, in1=xr, op=ALU.add)
                    nc.vector.tensor_scalar(
                        out=v, in0=v, scalar1=1e-30, scalar2=None, op0=ALU.max
                    )
                    nc.vector.reciprocal_approx_fast(out=v2, in_=v)
                    nc.vector.tensor_tensor(out=v, in0=xi, in1=v2, op=ALU.mult)
                    nc.scalar.activation(out=v, in_=v, func=AF.Arctan)
                    ph = tmp.tile([128, CH], F32, tag="tD", name=f"gph_{t}_{p}")
                    nc.vector.tensor_scalar(
                        out=ph, in0=v, scalar1=2.0, scalar2=None, op0=ALU.mult
                    )
                    nc.tensor.matmul(
                        sc_ps, gw_sb[:, p, :], amp, start=(p == 0), stop=False
                    )
                    nc.tensor.matmul(
                        sc_ps, gw_sb[:, KD + p, :], ph, start=False, stop=(p == KD - 1)
                    )
                sc_sb = scp.tile([8, CH], F32, tag="sc", bufs=1, name=f"scsb_{t}")
                nc.vector.tensor_scalar(
                    out=sc_sb, in0=sc_ps, scalar1=gb_sb[:, 0:1], scalar2=None,
                    op0=ALU.add,
                )
                for g4 in range(4):
                    tp_ps = pp.tile([128, 8], F32, tag="g", name=f"tpps_{t}_{g4}")
                    nc.tensor.transpose(
                        tp_ps, sc_sb[:, g4 * 128:(g4 + 1) * 128], ident[0:8, 0:8]
                    )
                    nc.scalar.copy(out=scores_t[:, t * 4 + g4, :], in_=tp_ps)
                # per-chunk softmax / top-1 weight (expert 0 = ours)
                gsl = slice(t * 4, (t + 1) * 4)
                nc.scalar.activation(
                    out=e_t[:, gsl, :], in_=scores_t[:, gsl, :], func=AF.Exp
                )
                nc.vector.tensor_reduce(
                    out=mx[:, gsl], in_=scores_t[:, gsl, :],
                    axis=mybir.AxisListType.X, op=ALU.max,
                )
                nc.vector.tensor_reduce(
                    out=sm[:, gsl], in_=e_t[:, gsl, :],
                    axis=mybir.AxisListType.X, op=ALU.add,
                )
                nc.vector.reciprocal_approx_fast(out=rs[:, gsl], in_=sm[:, gsl])
                nc.vector.tensor_tensor(
                    out=pe[:, gsl], in0=e_t[:, gsl, 0], in1=rs[:, gsl], op=ALU.mult
                )
                nc.vector.tensor_tensor(
                    out=msk[:, gsl], in0=scores_t[:, gsl, 0], in1=mx[:, gsl],
                    op=ALU.is_ge,
                )
                nc.vector.tensor_tensor(
                    out=w_pt[:, gsl], in0=pe[:, gsl], in1=msk[:, gsl], op=ALU.mult
                )
                wt_ps = pp.tile([4, 128], F32, tag="g", name=f"wtps_{t}")
                nc.tensor.transpose(wt_ps, w_pt[:, gsl], ident)
                w16c = scp.tile([4, 128], F32, tag="w16", name=f"w16c_{t}")
                nc.scalar.copy(out=w16c, in_=wt_ps)
                nc.sync.dma_start(out=w_scr[gsl, :], in_=w16c)
                wb_t = wbc.tile([128, CH], F32, tag=f"wb{t}", name=f"wb_{t}")
                for g4 in range(4):
                    g = t * 4 + g4
                    row = w_scr[g:g + 1, :]
                    bcast = bass.AP(
                        tensor=row.tensor, offset=row.offset,
                        ap=[[0, 128]] + list(row.ap[1:]),
                    )
                    nc.sync.dma_start(
                        out=wb_t[:, g4 * 128:(g4 + 1) * 128], in_=bcast
                    )
                return {"xrf": xrf, "xif": xif, "xr_pk": xr_pk, "xi_pk": xi_pk, "wb": wb_t, "tok": tok}

            def emit_experts(t, st):
                tok = st["tok"]
                wb_t = st["wb"]
                xrb_pk = xb.tile([128, 4, CH], BF16, tag="xrb", name=f"xrb_{t}")
                nc.vector.tensor_copy(out=xrb_pk, in_=st["xr_pk"])
                xib_pk = xb.tile([128, 4, CH], BF16, tag="xib", name=f"xib_{t}")
                nc.vector.tensor_copy(out=xib_pk, in_=st["xi_pk"])
                xnb_pk = xb.tile([128, 4, CH], BF16, tag="xnb", name=f"xnb_{t}")
                nc.vector.tensor_scalar(
                    out=xnb_pk, in0=st["xi_pk"], scalar1=-1.0, scalar2=None,
                    op0=ALU.mult,
                )
                xrb = [xrb_pk[:, p, :] for p in range(KD)]
                xib = [xib_pk[:, p, :] for p in range(KD)]
                xnb = [xnb_pk[:, p, :] for p in range(KD)]

                hrb, hib, hnb = [], [], []
                for m in range(KH):
                    msl = bass.ts(m, 128)
                    ps_hr = pp.tile([128, CH], F32, tag="hr", name=f"pshr_{t}_{m}")
                    for k in range(KD):
                        nc.tensor.matmul(
                            ps_hr, w1r_bf[k][:, msl], xrb[k],
                            start=(k == 0), stop=False,
                        )
                        nc.tensor.matmul(
                            ps_hr, w1i_bf[k][:, msl], xnb[k],
                            start=False, stop=(k == KD - 1),
                        )
                    ps_hi = pp.tile([128, CH], F32, tag="hi", name=f"pshi_{t}_{m}")
                    for k in range(KD):
                        nc.tensor.matmul(
                            ps_hi, w1i_bf[k][:, msl], xrb[k],
                            start=(k == 0), stop=False,
                        )
                        nc.tensor.matmul(
                            ps_hi, w1r_bf[k][:, msl], xib[k],
                            start=False, stop=(k == KD - 1),
                        )
                    # ComplexModReLU. Move (psum + b1) to SBUF on ACT first so
                    # the PSUM banks free fast and the PE never stalls.
                    b1r_m = b1r_sb[:, m:m + 1]
                    b1i_m = b1i_sb[:, m:m + 1]
                    mb_m = modb_sb[:, m:m + 1]
                    hrf = tmp.tile([128, CH], F32, tag="tE", name=f"hrf_{t}_{m}")
                    nc.scalar.activation(
                        out=hrf, in_=ps_hr, func=AF.Identity, bias=b1r_m
                    )
                    hif = tmp.tile([128, CH], F32, tag="tF", name=f"hif_{t}_{m}")
                    nc.scalar.activation(
                        out=hif, in_=ps_hi, func=AF.Identity, bias=b1i_m
                    )
                    v1 = tmp.tile([128, CH], F32, tag="tA", name=f"mv1_{t}_{m}")
                    nc.scalar.activation(out=v1, in_=hrf, func=AF.Square)
                    v2 = tmp.tile([128, CH], F32, tag="tB", name=f"mv2_{t}_{m}")
                    nc.scalar.activation(out=v2, in_=hif, func=AF.Square)
                    nc.gpsimd.tensor_tensor(out=v1, in0=v1, in1=v2, op=ALU.add)
                    nc.scalar.activation(out=v1, in_=v1, func=AF.Sqrt, bias=eps_sb)
                    nc.scalar.activation(out=v2, in_=v1, func=AF.Relu, bias=mb_m)
                    q = tmp.tile([128, CH], F32, tag="tC", name=f"mq_{t}_{m}")
                    nc.vector.reciprocal_approx_fast(out=q, in_=v1)
                    nc.vector.tensor_tensor(out=v2, in0=v2, in1=q, op=ALU.mult)
                    h_r = hp.tile([128, CH], BF16, tag=f"hr{m}", name=f"hr_{t}_{m}")
                    nc.vector.tensor_tensor(out=h_r, in0=hrf, in1=v2, op=ALU.mult)
                    h_i = hp.tile([128, CH], BF16, tag=f"hi{m}", name=f"hi_{t}_{m}")
                    nc.vector.tensor_tensor(out=h_i, in0=hif, in1=v2, op=ALU.mult)
                    h_n = hp.tile([128, CH], BF16, tag=f"hn{m}", name=f"hn_{t}_{m}")
                    nc.vector.tensor_scalar(
                        out=h_n, in0=h_i, scalar1=-1.0, scalar2=None, op0=ALU.mult
                    )
                    hrb.append(h_r)
                    hib.append(h_i)
                    hnb.append(h_n)

                for m4 in range(MD):
                    msl = bass.ts(m4, 128)
                    ps_or = pp.tile([128, CH], F32, tag="or", bufs=1, name=f"psor_{t}_{m4}")
                    for k in range(KH):
                        nc.tensor.matmul(
                            ps_or, w2r_bf[k][:, msl], hrb[k],
                            start=(k == 0), stop=False,
                        )
                        nc.tensor.matmul(
                            ps_or, w2i_bf[k][:, msl], hnb[k],
                            start=False, stop=(k == KH - 1),
                        )
                    ps_oi = pp.tile([128, CH], F32, tag="oi", bufs=1, name=f"psoi_{t}_{m4}")
                    for k in range(KH):
                        nc.tensor.matmul(
                            ps_oi, w2i_bf[k][:, msl], hrb[k],
                            start=(k == 0), stop=False,
                        )
                        nc.tensor.matmul(
                            ps_oi, w2r_bf[k][:, msl], hib[k],
                            start=False, stop=(k == KH - 1),
                        )
                    o_r = op.tile([128, CH], F32, tag="osr", name=f"or_{t}_{m4}")
                    nc.vector.scalar_tensor_tensor(
                        out=o_r, in0=ps_or, scalar=b2r_sb[:, m4:m4 + 1],
                        in1=wb_t, op0=ALU.add, op1=ALU.mult,
                    )
                    nc.gpsimd.dma_start(
                        out=out_r[m4 * 128:(m4 + 1) * 128, tok], in_=o_r
                    )
                    o_i = op.tile([128, CH], F32, tag="osi", name=f"oi_{t}_{m4}")
                    nc.vector.scalar_tensor_tensor(
                        out=o_i, in0=ps_oi, scalar=b2i_sb[:, m4:m4 + 1],
                        in1=wb_t, op0=ALU.add, op1=ALU.mult,
                    )
                    nc.gpsimd.dma_start(
                        out=out_i[m4 * 128:(m4 + 1) * 128, tok], in_=o_i
                    )

            states = {}
            for t in range(NCH + 1):
                if t < NCH:
                    states[t] = emit_gating(t)
                if t >= 1:
                    emit_experts(t - 1, states.pop(t - 1))

    nc.compile()
    return nc


def kernel(**inputs):
    global LAST_RESULT
    f32 = lambda a: np.ascontiguousarray(np.asarray(a, dtype=np.float32))
    xr = f32(inputs["x_real"]).reshape(NT, D).T.copy()
    xi = f32(inputs["x_imag"]).reshape(NT, D).T.copy()
    gW = f32(inputs["gate_W"])
    gb = f32(inputs["gate_b"])
    W1r, W1i = f32(inputs["W1r"]), f32(inputs["W1i"])
    W2r, W2i = f32(inputs["W2r"]), f32(inputs["W2i"])
    b1r, b1i = f32(inputs["b1r"]), f32(inputs["b1i"])
    modb = f32(inputs["mod_b"])
    b2r, b2i = f32(inputs["b2r"]), f32(inputs["b2i"])

    if "nc" not in _CACHE:
        _CACHE["nc"] = _build_nc()
    nc = _CACHE["nc"]

    in_maps = []
    for c in range(E):
        perm = [c] + [e for e in range(E) if e != c]
        gWp = np.ascontiguousarray(
            gW[:, perm].reshape(8, 128, 8).transpose(1, 0, 2)
        )
        in_maps.append({
            "xrT": xr, "xiT": xi,
            "gWp": gWp,
            "gb": np.ascontiguousarray(gb[perm].reshape(8, 1)),
            "b1r": np.ascontiguousarray(b1r[c].reshape(KH, 128).T),
            "b1i": np.ascontiguousarray(b1i[c].reshape(KH, 128).T),
            "modb": np.ascontiguousarray(modb[c].reshape(KH, 128).T),
            "b2r": np.ascontiguousarray(b2r[c].reshape(MD, 128).T),
            "b2i": np.ascontiguousarray(b2i[c].reshape(MD, 128).T),
            "W1r": np.ascontiguousarray(W1r[c]),
            "W1i": np.ascontiguousarray(W1i[c]),
            "W2r": np.ascontiguousarray(W2r[c]),
            "W2i": np.ascontiguousarray(W2i[c]),
        })

    res = run_bass_kernel_spmd(nc, in_maps, list(range(E)))
    LAST_RESULT = res
    acc_r = np.zeros((D, NT), np.float32)
    acc_i = np.zeros((D, NT), np.float32)
    for c in range(E):
        acc_r += res.results[c]["out_r"]
        acc_i += res.results[c]["out_i"]
    out_r = np.ascontiguousarray(acc_r.T).reshape(B, S, D)
    out_i = np.ascontiguousarray(acc_i.T).reshape(B, S, D)
    return out_r, out_i
